# revision 9
# baseline (speedup 1.0000x reference)
"""Trainium2 Bass kernel for LoopCoderAttention (sparse_attention).

Head-sharded tensor parallelism over 8 NeuronCores:
  core c owns query heads {2c, 2c+1} and KV head c//2.
All on-device tensors live in transposed [feature, token] layout so every
matmul contracts along the partition dim with zero on-device transposes
(except v, which needs one PE transpose per 128-tile).

o_proj: a 2MB AllToAll reshards attention output from head-sharded to
token-sharded; each core then runs the full 2048-deep contraction for its
256-token slice (the "all-reduce" happens inside the matmul accumulation).
"""
import sys
sys.path.insert(0, '/opt/trn_rl_repo')
import numpy as np
import concourse.bass as bass
import concourse.mybir as mybir
import concourse.tile as tile
from concourse import bacc
from concourse.bass_utils import run_bass_kernel_spmd

T = 2048
HID = 2048
HQ = 16
HK = 4
D = 128
WIN = 64
THETA = 10000.0
SCALE = D ** -0.5
NCORES = 8
TCH = 512                 # t-chunk (matmul free dim)
NCH = T // TCH            # 4 chunks
KT = HID // 128           # 16 k-tiles for 2048-deep contractions
ST = T // 128             # 16 s-tiles
TSL = T // NCORES         # 256-token output slice per core
MASKV = -1e9

F32 = mybir.dt.float32
F32R = mybir.dt.float32r
AF = mybir.ActivationFunctionType

_CACHE = {}


def _build():
    nc = bacc.Bacc("TRN2", target_bir_lowering=False, debug=False,
                   num_devices=NCORES)
    HST = nc.dram_tensor("HST", [HID, T], F32, kind="ExternalInput").ap()
    WQKV = nc.dram_tensor("WQKV", [HID, 512], F32, kind="ExternalInput").ap()
    KGT = nc.dram_tensor("KGT", [D, T], F32, kind="ExternalInput").ap()
    VG = nc.dram_tensor("VG", [T, D], F32, kind="ExternalInput").ap()
    WO = nc.dram_tensor("WO", [HID, HID], F32, kind="ExternalInput").ap()
    WG = nc.dram_tensor("WG", [D, 2], F32, kind="ExternalInput").ap()
    BG = nc.dram_tensor("BG", [1, 2], F32, kind="ExternalInput").ap()
    CSF = nc.dram_tensor("CSF", [128, T], F32, kind="ExternalInput").ap()
    SNF = nc.dram_tensor("SNF", [128, T], F32, kind="ExternalInput").ap()
    ONES = nc.dram_tensor("ONES", [128, 1], F32, kind="ExternalInput").ap()
    IDN = nc.dram_tensor("IDN", [128, 128], F32, kind="ExternalInput").ap()
    MASKC = nc.dram_tensor("MASKC", [128, 896], F32, kind="ExternalInput").ap()
    MASKL = nc.dram_tensor("MASKL", [128, 1152], F32, kind="ExternalInput").ap()
    OUT = nc.dram_tensor("OUT", [TSL, HID], F32, kind="ExternalOutput").ap()

    with tile.TileContext(nc) as tc:
        # pools are a strict stack: creation order is the reverse of the
        # release order at each phase boundary
        const = tc.alloc_tile_pool(name="const", bufs=1)
        dram = tc.alloc_tile_pool(name="dram", bufs=1, space="DRAM")
        aoutp = tc.alloc_tile_pool(name="aoutp", bufs=3)
        work = tc.alloc_tile_pool(name="work", bufs=1)
        expp = tc.alloc_tile_pool(name="expp", bufs=4)
        ropet = tc.alloc_tile_pool(name="ropet", bufs=3)
        rcpp = tc.alloc_tile_pool(name="rcpp", bufs=4)
        bcp = tc.alloc_tile_pool(name="bcp", bufs=3)
        combp = tc.alloc_tile_pool(name="combp", bufs=3)
        wqkvp = tc.alloc_tile_pool(name="wqkvp", bufs=1)
        chunkp = tc.alloc_tile_pool(name="chunkp", bufs=2)
        hsp = tc.alloc_tile_pool(name="hsp", bufs=4)
        ps1 = tc.alloc_tile_pool(name="ps1", bufs=4, space="PSUM")

        # ---- constants in ----
        kgt_sb = const.tile([D, T], F32R)
        nc.sync.dma_start(out=kgt_sb[:], in_=KGT.bitcast(F32R))
        vg_sb = const.tile([128, ST, D], F32R)
        nc.sync.dma_start(out=vg_sb[:],
                          in_=VG.rearrange("(s p) d -> p s d", p=128).bitcast(F32R))
        wg_sb = const.tile([D, 2], F32R)
        nc.sync.dma_start(out=wg_sb[:], in_=WG.bitcast(F32R))
        bg_sb = const.tile([1, 2], F32)
        nc.sync.dma_start(out=bg_sb[:], in_=BG)
        csf_sb = const.tile([128, T], F32)
        nc.sync.dma_start(out=csf_sb[:], in_=CSF)
        snf_sb = const.tile([128, T], F32)
        nc.sync.dma_start(out=snf_sb[:], in_=SNF)
        ones_sb = const.tile([128, 1], F32R)
        nc.sync.dma_start(out=ones_sb[:], in_=ONES.bitcast(F32R))
        idn_sb = const.tile([128, 128], F32)
        nc.sync.dma_start(out=idn_sb[:], in_=IDN)
        maskc_sb = const.tile([128, 896], F32)
        nc.sync.dma_start(out=maskc_sb[:], in_=MASKC)
        maskl_sb = const.tile([128, 1152], F32)
        nc.sync.dma_start(out=maskl_sb[:], in_=MASKL)

        wqkv_sb = wqkvp.tile([128, KT, 512], F32R)
        nc.sync.dma_start(out=wqkv_sb[:],
                          in_=WQKV.rearrange("(k p) c -> p k c", p=128).bitcast(F32R))

        # ---- persistent work tiles (through attention) ----
        qrot = work.tile([128, 2, T], F32R)
        krot = work.tile([128, T], F32R)
        vcur = work.tile([128, ST, D], F32R)   # current v in [s, d] tiles
        gate = work.tile([1, 8 * TCH], F32)    # slot 2n+h along free dim

        a2ai = dram.tile([NCORES, 2 * D, TSL], F32)
        a2ao = dram.tile([NCORES, 2 * D, TSL], F32)

        def rope_chunk(dst_full, src, n):
            """dst_full[:, n*TCH:...] = neox-rope of chunk tile src [128, TCH].

            rot = src * [cos;cos] + rot90(src) * [-sin;sin], where rot90 swaps
            the two 64-partition halves (built with two SBUF->SBUF DMAs since
            DVE ops require matching base partitions).
            """
            sl = bass.ds(n * TCH, TCH)
            sr = ropet.tile([128, TCH], F32, tag="ropesr", name=f"sr{n}")
            nc.sync.dma_start(out=sr[0:64, :], in_=src[64:128, :])
            nc.sync.dma_start(out=sr[64:128, :], in_=src[0:64, :])
            ta = ropet.tile([128, TCH], F32, tag="ropetmp", name=f"ra{n}")
            tb = ropet.tile([128, TCH], F32, tag="ropetmp", name=f"rb{n}")
            nc.vector.tensor_mul(ta[:], src[:], csf_sb[:, sl])
            nc.vector.tensor_mul(tb[:], sr[:], snf_sb[:, sl])
            nc.vector.tensor_add(dst_full[:, sl], ta[:], tb[:])

        # ================= phase 1: qkvT = wqkv^T @ hsT =================
        for n in range(NCH):
            pss = [ps1.tile([128, TCH], F32, tag="ps1t", name=f"ps1_{n}_{m}")
                   for m in range(4)]
            for k in range(KT):
                hs_t = hsp.tile([128, TCH], F32R)
                nc.sync.dma_start(
                    out=hs_t[:],
                    in_=HST[k * 128:(k + 1) * 128,
                            n * TCH:(n + 1) * TCH].bitcast(F32R))
                for m in range(4):
                    nc.tensor.matmul(pss[m][:],
                                     wqkv_sb[:, k, m * 128:(m + 1) * 128],
                                     hs_t[:],
                                     start=(k == 0), stop=(k == KT - 1))
            sl = bass.ds(n * TCH, TCH)
            q0c = chunkp.tile([128, TCH], F32, tag="q0c")
            q1c = chunkp.tile([128, TCH], F32, tag="q1c")
            kc = chunkp.tile([128, TCH], F32, tag="kc")
            vc = chunkp.tile([128, TCH], F32, tag="vc")
            nc.scalar.activation(q0c[:], pss[0][:], AF.Copy)
            nc.scalar.activation(q1c[:], pss[1][:], AF.Copy)
            nc.scalar.activation(kc[:], pss[2][:], AF.Copy)
            nc.vector.tensor_copy(vc[:], pss[3][:])

            rope_chunk(qrot[:, 0, :], q0c, n)
            rope_chunk(qrot[:, 1, :], q1c, n)
            rope_chunk(krot, kc, n)

            # transpose v tiles of this chunk: vcur[s] = vc[:, j*128:...]^T
            for j in range(4):
                s = 4 * n + j
                pt = ps1.tile([128, 128], F32, tag="ps1t", name=f"pt{s}")
                nc.tensor.transpose(pt[:], vc[:, j * 128:(j + 1) * 128],
                                    idn_sb[:])
                nc.vector.tensor_copy(vcur[:, s, :], pt[:])

            # gate for this chunk (both heads)
            for h in range(2):
                r = 2 * n + h
                gp = ps1.tile([1, TCH], F32, tag="ps1g", name=f"gp{r}")
                nc.tensor.matmul(gp[:], wg_sb[:, h:h + 1], qrot[:, h, sl],
                                 start=True, stop=True)
                nc.scalar.activation(gate[0:1, r * TCH:(r + 1) * TCH], gp[:],
                                     AF.Sigmoid, bias=bg_sb[0:1, h:h + 1])

        ps1.release()
        hsp.release()
        chunkp.release()
        wqkvp.release()

        psqk = tc.alloc_tile_pool(name="psqk", bufs=2, space="PSUM")
        pspv = tc.alloc_tile_pool(name="pspv", bufs=3, space="PSUM")
        pssm = tc.alloc_tile_pool(name="pssm", bufs=3, space="PSUM")

        # ============ phase 2: attention (global + local) ============
        for h in range(2):
            for n in range(NCH):
                sl = bass.ds(n * TCH, TCH)
                q_ap = qrot[:, h, sl]

                def pass_(kT_ap, v_ap, s_tiles, mask_ap_of, pfx):
                    pv = pspv.tile([128, TCH], F32, tag="pv", name=f"pv{pfx}")
                    sm = pssm.tile([1, TCH], F32, tag="sm", name=f"sm{pfx}")
                    first = True
                    for s in s_tiles:
                        qk = psqk.tile([128, TCH], F32, tag="qk",
                                       name=f"qk{pfx}_{s}")
                        nc.tensor.matmul(qk[:], kT_ap[:, s * 128:(s + 1) * 128],
                                         q_ap, start=True, stop=True)
                        m_ap = mask_ap_of(s)
                        if m_ap is not None:
                            nc.vector.tensor_add(qk[:], qk[:], m_ap)
                        ex = expp.tile([128, TCH], F32R, tag="ex", name=f"ex{pfx}_{s}")
                        nc.scalar.activation(ex[:], qk[:], AF.Exp, scale=SCALE)
                        last = (s == s_tiles[-1])
                        nc.tensor.matmul(pv[:], v_ap[:, s, :], ex[:],
                                         start=first, stop=last)
                        nc.tensor.matmul(sm[:], ones_sb[:], ex[:],
                                         start=first, stop=last)
                        first = False
                    return pv, sm

                # global pass over cached KV: causal mask on diagonal tiles
                gs = list(range(0, 4 * n + 4))

                def gmask(s, n=n):
                    j = s - 4 * n
                    if j < 0:
                        return None
                    off = (3 - j) * 128
                    return maskc_sb[:, off:off + TCH]

                pv_g, sm_g = pass_(kgt_sb, vg_sb, gs, gmask, f"g{h}{n}")

                # local pass over current KV: sliding-window band masks
                ls = [s for s in range(4 * n - 1, 4 * n + 4) if s >= 0]

                def lmask(s, n=n):
                    jj = s - (4 * n - 1)
                    off = 640 - 128 * jj
                    return maskl_sb[:, off:off + TCH]

                pv_l, sm_l = pass_(krot, vcur, ls, lmask, f"l{h}{n}")

                # normalize + gate combine
                r = 2 * n + h
                rg = rcpp.tile([1, TCH], F32, tag="rcp", name=f"rg{r}")
                rl = rcpp.tile([1, TCH], F32, tag="rcp", name=f"rl{r}")
                nc.vector.reciprocal(rg[:], sm_g[:])
                nc.vector.reciprocal(rl[:], sm_l[:])
                ag = rcpp.tile([1, TCH], F32, tag="rcp", name=f"ag{r}")
                al = rcpp.tile([1, TCH], F32, tag="rcp", name=f"al{r}")
                gsl = gate[0:1, r * TCH:(r + 1) * TCH]
                nc.vector.tensor_mul(ag[:], gsl, rg[:])
                nc.vector.tensor_mul(al[:], gsl, rl[:])
                nc.vector.tensor_sub(al[:], rl[:], al[:])
                bg_t = bcp.tile([128, TCH], F32, tag="bcast", name=f"bg_t{r}")
                bl_t = bcp.tile([128, TCH], F32, tag="bcast", name=f"bl_t{r}")
                nc.gpsimd.partition_broadcast(bg_t[:], ag[:])
                nc.gpsimd.partition_broadcast(bl_t[:], al[:])
                t1 = combp.tile([128, TCH], F32, tag="comb", name=f"t1{r}")
                t2 = combp.tile([128, TCH], F32, tag="comb", name=f"t2{r}")
                ao = aoutp.tile([128, TCH], F32, tag="aout", name=f"ao{r}")
                nc.vector.tensor_mul(t1[:], pv_g[:], bg_t[:])
                nc.vector.tensor_mul(t2[:], pv_l[:], bl_t[:])
                nc.vector.tensor_add(ao[:], t1[:], t2[:])

                # ship finished 256-col blocks to a2a staging
                for i, c in enumerate((2 * n, 2 * n + 1)):
                    nc.sync.dma_start(out=a2ai[c, h * D:(h + 1) * D, :],
                                      in_=ao[:, i * TSL:(i + 1) * TSL])

        pssm.release()
        pspv.release()
        psqk.release()
        combp.release()
        bcp.release()
        rcpp.release()
        ropet.release()
        expp.release()
        work.release()
        aoutp.release()

        # ========= phase 3: all-to-all reshard (heads -> tokens) =========
        nc.gpsimd.collective_compute(
            "AllToAll", mybir.AluOpType.bypass,
            replica_groups=[list(range(NCORES))],
            ins=[a2ai[:].opt()], outs=[a2ao[:].opt()])

        opool = tc.alloc_tile_pool(name="opool", bufs=1)
        wop = tc.alloc_tile_pool(name="wop", bufs=4)
        osb = tc.alloc_tile_pool(name="osb", bufs=4)
        pso = tc.alloc_tile_pool(name="pso", bufs=8, space="PSUM")

        afull = opool.tile([128, KT, TSL], F32R)
        nc.sync.dma_start(
            out=afull[:],
            in_=a2ao[:].rearrange("c p n -> (c p) n")
                       .rearrange("(k p) n -> p k n", p=128).bitcast(F32R))

        # ============ phase 4: o_proj for our token slice ============
        pss2 = [[pso.tile([128, TCH], F32, tag="po", name=f"po_{tt}_{e}")
                 for e in range(NCH)] for tt in range(2)]
        for k in range(KT):
            wo_t = wop.tile([128, HID], F32R, tag="wo", name=f"wo{k}")
            nc.sync.dma_start(out=wo_t[:],
                              in_=WO[k * 128:(k + 1) * 128, :].bitcast(F32R))
            for tt in range(2):
                for e in range(NCH):
                    nc.tensor.matmul(pss2[tt][e][:],
                                     afull[:, k, tt * 128:(tt + 1) * 128],
                                     wo_t[:, e * TCH:(e + 1) * TCH],
                                     start=(k == 0), stop=(k == KT - 1))
        for tt in range(2):
            for e in range(NCH):
                ot = osb.tile([128, TCH], F32, tag="ot", name=f"ot{tt}_{e}")
                nc.vector.tensor_copy(ot[:], pss2[tt][e][:])
                nc.sync.dma_start(
                    out=OUT[tt * 128:(tt + 1) * 128,
                            e * TCH:(e + 1) * TCH],
                    in_=ot[:])
        pso.release()
        osb.release()
        wop.release()
        opool.release()
        dram.release()
        const.release()

    nc.compile()
    return nc


def _host_prep(hidden_states, positions, k_global, v_global, w_qkv, w_o,
               w_gate, b_gate):
    """Layout-only host transforms + constant tables -> per-core in_maps."""
    f32 = np.float32
    hs = np.ascontiguousarray(np.asarray(hidden_states, f32))
    pos = np.asarray(positions)
    kg = np.asarray(k_global, f32)
    vg = np.asarray(v_global, f32)
    wqkv = np.asarray(w_qkv, f32)
    wo = np.ascontiguousarray(np.asarray(w_o, f32))
    wg = np.asarray(w_gate, f32)
    bg = np.asarray(b_gate, f32)

    hst = np.ascontiguousarray(hs.T)

    half = D // 2
    inv_freq = (THETA ** (-np.arange(half, dtype=f32) / half)).astype(f32)
    ang = pos.astype(f32)[:, None] * inv_freq[None, :]
    cos_t = np.cos(ang).astype(f32).T       # [64, T]
    sin_t = np.sin(ang).astype(f32).T
    csf = np.ascontiguousarray(np.concatenate([cos_t, cos_t], axis=0))
    snf = np.ascontiguousarray(np.concatenate([-sin_t, sin_t], axis=0))

    p = np.arange(128, dtype=np.int64)[:, None]
    # causal diag-band base mask: CM[j][p, x] = MASKC[p, x + (3-j)*128]
    yc = np.arange(896, dtype=np.int64)[None, :]
    maskc = np.where(yc - p - 384 >= 0, 0.0, MASKV).astype(f32)
    # local band base mask: LM[jj][p, x] = MASKL[p, x + 640 - 128*jj]
    yl = np.arange(1152, dtype=np.int64)[None, :]
    dl = yl - 512 - p
    maskl = np.where((dl >= 0) & (dl <= WIN), 0.0, MASKV).astype(f32)

    ones = np.ones((128, 1), f32)
    idn = np.eye(128, dtype=f32)

    in_maps = []
    for c in range(NCORES):
        g = c // 2
        wq = wqkv[:, 2 * c * D:(2 * c + 2) * D]
        wk = wqkv[:, HQ * D + g * D:HQ * D + (g + 1) * D]
        wv = wqkv[:, (HQ + HK) * D + g * D:(HQ + HK) * D + (g + 1) * D]
        in_maps.append({
            "HST": hst,
            "WQKV": np.ascontiguousarray(np.concatenate([wq, wk, wv], axis=1)),
            "KGT": np.ascontiguousarray(kg[:, g * D:(g + 1) * D].T),
            "VG": np.ascontiguousarray(vg[:, g * D:(g + 1) * D]),
            "WO": wo,
            "WG": np.ascontiguousarray(wg[:, 2 * c:2 * c + 2]),
            "BG": np.ascontiguousarray(bg[2 * c:2 * c + 2].reshape(1, 2)),
            "CSF": csf,
            "SNF": snf,
            "ONES": ones,
            "IDN": idn,
            "MASKC": maskc,
            "MASKL": maskl,
        })
    return in_maps


def kernel(**inputs):
    if "nc" not in _CACHE:
        _CACHE["nc"] = _build()
    nc = _CACHE["nc"]
    in_maps = _host_prep(**inputs)
    res = run_bass_kernel_spmd(nc, in_maps, core_ids=list(range(NCORES)))
    out = np.concatenate([res.results[c]["OUT"] for c in range(NCORES)], axis=0)
    return out.astype(np.float32)


# revision 12
# speedup vs baseline: 1.1939x; 1.1939x over previous
"""Trainium2 Bass kernel for LoopCoderAttention (sparse_attention).

Head-sharded tensor parallelism over 8 NeuronCores:
  core c owns query heads {2c, 2c+1} and KV head c//2.
All on-device tensors live in transposed [feature, token] layout so every
matmul contracts along the partition dim with zero on-device transposes
(except v, which needs one PE transpose per 128-tile).

o_proj: a 2MB AllToAll reshards attention output from head-sharded to
token-sharded; each core then runs the full 2048-deep contraction for its
256-token slice (the "all-reduce" happens inside the matmul accumulation).
"""
import sys
sys.path.insert(0, '/opt/trn_rl_repo')
import numpy as np
import concourse.bass as bass
import concourse.mybir as mybir
import concourse.tile as tile
from concourse import bacc
from concourse.bass_utils import run_bass_kernel_spmd

T = 2048
HID = 2048
HQ = 16
HK = 4
D = 128
WIN = 64
THETA = 10000.0
SCALE = D ** -0.5
NCORES = 8
TCH = 512                 # t-chunk (matmul free dim)
NCH = T // TCH            # 4 chunks
KT = HID // 128           # 16 k-tiles for 2048-deep contractions
ST = T // 128             # 16 s-tiles
TSL = T // NCORES         # 256-token output slice per core
MASKV = -1e9

F32 = mybir.dt.float32
F32R = mybir.dt.float32r
AF = mybir.ActivationFunctionType

_CACHE = {}


def _build():
    nc = bacc.Bacc("TRN2", target_bir_lowering=False, debug=False,
                   num_devices=NCORES)
    HST = nc.dram_tensor("HST", [HID, T], F32, kind="ExternalInput").ap()
    WQKV = nc.dram_tensor("WQKV", [HID, 512], F32, kind="ExternalInput").ap()
    KGT = nc.dram_tensor("KGT", [D, T], F32, kind="ExternalInput").ap()
    VG = nc.dram_tensor("VG", [T, D], F32, kind="ExternalInput").ap()
    WO = nc.dram_tensor("WO", [HID, HID], F32, kind="ExternalInput").ap()
    WG = nc.dram_tensor("WG", [D, 2], F32, kind="ExternalInput").ap()
    BG = nc.dram_tensor("BG", [1, 2], F32, kind="ExternalInput").ap()
    CSF = nc.dram_tensor("CSF", [128, T], F32, kind="ExternalInput").ap()
    SNF = nc.dram_tensor("SNF", [128, T], F32, kind="ExternalInput").ap()
    ONES = nc.dram_tensor("ONES", [128, 1], F32, kind="ExternalInput").ap()
    IDN = nc.dram_tensor("IDN", [128, 128], F32, kind="ExternalInput").ap()
    MASKC = nc.dram_tensor("MASKC", [128, 896], F32, kind="ExternalInput").ap()
    MASKL = nc.dram_tensor("MASKL", [128, 1152], F32, kind="ExternalInput").ap()
    OUT = nc.dram_tensor("OUT", [TSL, HID], F32, kind="ExternalOutput").ap()

    with tile.TileContext(nc) as tc:
        # pools are a strict stack: creation order is the reverse of the
        # release order at each phase boundary
        const = tc.alloc_tile_pool(name="const", bufs=1)
        dram = tc.alloc_tile_pool(name="dram", bufs=1, space="DRAM")
        aoutp = tc.alloc_tile_pool(name="aoutp", bufs=3)
        work = tc.alloc_tile_pool(name="work", bufs=1)
        expp = tc.alloc_tile_pool(name="expp", bufs=4)
        ropet = tc.alloc_tile_pool(name="ropet", bufs=3)
        rcpp = tc.alloc_tile_pool(name="rcpp", bufs=8)
        bcp = tc.alloc_tile_pool(name="bcp", bufs=3)
        combp = tc.alloc_tile_pool(name="combp", bufs=5)
        wqkvp = tc.alloc_tile_pool(name="wqkvp", bufs=1)
        chunkp = tc.alloc_tile_pool(name="chunkp", bufs=2)
        hsp = tc.alloc_tile_pool(name="hsp", bufs=4)
        ps1 = tc.alloc_tile_pool(name="ps1", bufs=4, space="PSUM")

        # ---- phase-1 constants first (critical path to first matmul) ----
        wqkv_sb = wqkvp.tile([128, KT, 512], F32R)
        wqkv_view = WQKV.rearrange("(k p) c -> p k c", p=128).bitcast(F32R)
        for k in range(KT):
            nc.sync.dma_start(out=wqkv_sb[:, k, :], in_=wqkv_view[:, k, :])
        csf_sb = const.tile([128, T], F32)
        nc.sync.dma_start(out=csf_sb[:], in_=CSF)
        snf_sb = const.tile([128, T], F32)
        nc.sync.dma_start(out=snf_sb[:], in_=SNF)
        idn_sb = const.tile([128, 128], F32)
        nc.sync.dma_start(out=idn_sb[:], in_=IDN)
        wg_sb = const.tile([D, 2], F32R)
        nc.sync.dma_start(out=wg_sb[:], in_=WG.bitcast(F32R))
        bg_sb = const.tile([1, 2], F32)
        nc.sync.dma_start(out=bg_sb[:], in_=BG)
        # attention-phase constants (scheduler fills DMA idle time)
        kgt_sb = const.tile([D, T], F32R)
        vg_sb = const.tile([128, ST, D], F32R)
        ones_sb = const.tile([128, 1], F32R)
        maskc_sb = const.tile([128, 896], F32)
        maskl_sb = const.tile([128, 1152], F32)

        # ---- persistent work tiles (through attention) ----
        qrot = work.tile([128, 2, T], F32R)
        krot = work.tile([128, T], F32R)
        vcur = work.tile([128, ST, D], F32R)   # current v in [s, d] tiles
        gate = work.tile([1, 8 * TCH], F32)    # slot 2n+h along free dim

        a2ai = dram.tile([NCORES, 2 * D, TSL], F32)
        a2ao = dram.tile([NCORES, 2 * D, TSL], F32)

        def rope_chunk(dst_full, src, n):
            """dst_full[:, n*TCH:...] = neox-rope of chunk tile src [128, TCH].

            rot = src * [cos;cos] + rot90(src) * [-sin;sin], where rot90 swaps
            the two 64-partition halves (built with two SBUF->SBUF DMAs since
            DVE ops require matching base partitions).
            """
            sl = bass.ds(n * TCH, TCH)
            sr = ropet.tile([128, TCH], F32, tag="ropesr", name=f"sr{n}")
            nc.sync.dma_start(out=sr[0:64, :], in_=src[64:128, :])
            nc.sync.dma_start(out=sr[64:128, :], in_=src[0:64, :])
            ta = ropet.tile([128, TCH], F32, tag="ropetmp", name=f"ra{n}")
            tb = ropet.tile([128, TCH], F32, tag="ropetmp", name=f"rb{n}")
            nc.vector.tensor_mul(ta[:], src[:], csf_sb[:, sl])
            nc.vector.tensor_mul(tb[:], sr[:], snf_sb[:, sl])
            nc.vector.tensor_add(dst_full[:, sl], ta[:], tb[:])

        # ================= phase 1: qkvT = wqkv^T @ hsT =================
        for n in range(NCH):
            pss = [ps1.tile([128, TCH], F32, tag="ps1t", name=f"ps1_{n}_{m}")
                   for m in range(4)]
            for k in range(KT):
                hs_t = hsp.tile([128, TCH], F32R)
                nc.sync.dma_start(
                    out=hs_t[:],
                    in_=HST[k * 128:(k + 1) * 128,
                            n * TCH:(n + 1) * TCH].bitcast(F32R))
                for m in range(4):
                    nc.tensor.matmul(pss[m][:],
                                     wqkv_sb[:, k, m * 128:(m + 1) * 128],
                                     hs_t[:],
                                     start=(k == 0), stop=(k == KT - 1))
            sl = bass.ds(n * TCH, TCH)
            q0c = chunkp.tile([128, TCH], F32, tag="q0c")
            q1c = chunkp.tile([128, TCH], F32, tag="q1c")
            kc = chunkp.tile([128, TCH], F32, tag="kc")
            vc = chunkp.tile([128, TCH], F32, tag="vc")
            nc.scalar.activation(q0c[:], pss[0][:], AF.Copy)
            nc.scalar.activation(q1c[:], pss[1][:], AF.Copy)
            nc.scalar.activation(kc[:], pss[2][:], AF.Copy)
            nc.vector.tensor_copy(vc[:], pss[3][:])

            rope_chunk(qrot[:, 0, :], q0c, n)
            rope_chunk(qrot[:, 1, :], q1c, n)
            rope_chunk(krot, kc, n)

            # transpose v tiles of this chunk: vcur[s] = vc[:, j*128:...]^T
            for j in range(4):
                s = 4 * n + j
                pt = ps1.tile([128, 128], F32, tag="ps1t", name=f"pt{s}")
                nc.tensor.transpose(pt[:], vc[:, j * 128:(j + 1) * 128],
                                    idn_sb[:])
                nc.vector.tensor_copy(vcur[:, s, :], pt[:])

            # gate for this chunk (both heads)
            for h in range(2):
                r = 2 * n + h
                gp = ps1.tile([1, TCH], F32, tag="ps1g", name=f"gp{r}")
                nc.tensor.matmul(gp[:], wg_sb[:, h:h + 1], qrot[:, h, sl],
                                 start=True, stop=True)
                nc.scalar.activation(gate[0:1, r * TCH:(r + 1) * TCH], gp[:],
                                     AF.Sigmoid, bias=bg_sb[0:1, h:h + 1])

        nc.sync.dma_start(out=kgt_sb[:], in_=KGT.bitcast(F32R))
        nc.sync.dma_start(out=vg_sb[:],
                          in_=VG.rearrange("(s p) d -> p s d", p=128).bitcast(F32R))
        nc.sync.dma_start(out=ones_sb[:], in_=ONES.bitcast(F32R))
        nc.sync.dma_start(out=maskc_sb[:], in_=MASKC)
        nc.sync.dma_start(out=maskl_sb[:], in_=MASKL)

        ps1.release()
        hsp.release()
        chunkp.release()
        wqkvp.release()

        psqk = tc.alloc_tile_pool(name="psqk", bufs=4, space="PSUM")
        pspv = tc.alloc_tile_pool(name="pspv", bufs=2, space="PSUM")
        pssm = tc.alloc_tile_pool(name="pssm", bufs=2, space="PSUM")

        # ============ phase 2: attention (global + local) ============
        for h in range(2):
            for n in range(NCH):
                sl = bass.ds(n * TCH, TCH)
                q_ap = qrot[:, h, sl]

                def pass_(kT_ap, v_ap, s_tiles, mask_ap_of, pfx):
                    pv = pspv.tile([128, TCH], F32, tag="pv", name=f"pv{pfx}")
                    sm = pssm.tile([1, TCH], F32, tag="sm", name=f"sm{pfx}")
                    first = True
                    for s in s_tiles:
                        qk = psqk.tile([128, TCH], F32, tag="qk",
                                       name=f"qk{pfx}_{s}")
                        nc.tensor.matmul(qk[:], kT_ap[:, s * 128:(s + 1) * 128],
                                         q_ap, start=True, stop=True)
                        m_ap = mask_ap_of(s)
                        if m_ap is not None:
                            nc.vector.tensor_add(qk[:], qk[:], m_ap)
                        ex = expp.tile([128, TCH], F32R, tag="ex", name=f"ex{pfx}_{s}")
                        nc.scalar.activation(ex[:], qk[:], AF.Exp, scale=SCALE)
                        last = (s == s_tiles[-1])
                        nc.tensor.matmul(pv[:], v_ap[:, s, :], ex[:],
                                         start=first, stop=last)
                        nc.tensor.matmul(sm[:], ones_sb[:], ex[:],
                                         start=first, stop=last)
                        first = False
                    return pv, sm

                # global pass over cached KV: causal mask on diagonal tiles
                gs = list(range(0, 4 * n + 4))

                def gmask(s, n=n):
                    j = s - 4 * n
                    if j < 0:
                        return None
                    off = (3 - j) * 128
                    return maskc_sb[:, off:off + TCH]

                pv_g, sm_g = pass_(kgt_sb, vg_sb, gs, gmask, f"g{h}{n}")

                # local pass over current KV: sliding-window band masks
                ls = [s for s in range(4 * n - 1, 4 * n + 4) if s >= 0]

                def lmask(s, n=n):
                    jj = s - (4 * n - 1)
                    off = 640 - 128 * jj
                    return maskl_sb[:, off:off + TCH]

                pv_l, sm_l = pass_(krot, vcur, ls, lmask, f"l{h}{n}")

                # drain psum fast via ScalarE so PE never stalls on slots
                r = 2 * n + h
                pg = combp.tile([128, TCH], F32, tag="comb", name=f"pg{r}")
                pl = combp.tile([128, TCH], F32, tag="comb", name=f"pl{r}")
                sg = rcpp.tile([1, TCH], F32, tag="rcp", name=f"sg{r}")
                sl_ = rcpp.tile([1, TCH], F32, tag="rcp", name=f"sl{r}")
                nc.scalar.activation(pg[:], pv_g[:], AF.Copy)
                nc.scalar.activation(pl[:], pv_l[:], AF.Copy)
                nc.scalar.activation(sg[:], sm_g[:], AF.Copy)
                nc.scalar.activation(sl_[:], sm_l[:], AF.Copy)
                # a_g = gate/sum_g ; a_l = (1-gate)/sum_l
                gsl = gate[0:1, r * TCH:(r + 1) * TCH]
                ag = rcpp.tile([1, TCH], F32, tag="rcp", name=f"ag{r}")
                al = rcpp.tile([1, TCH], F32, tag="rcp", name=f"al{r}")
                g1 = rcpp.tile([1, TCH], F32, tag="rcp", name=f"g1{r}")
                nc.vector.tensor_scalar(g1[:], gsl, -1.0, 1.0,
                                        mybir.AluOpType.mult,
                                        mybir.AluOpType.add)
                rg = rcpp.tile([1, TCH], F32, tag="rcp", name=f"rg{r}")
                rl = rcpp.tile([1, TCH], F32, tag="rcp", name=f"rl{r}")
                nc.vector.reciprocal_approx_fast(rg[:], sg[:])
                nc.vector.reciprocal_approx_fast(rl[:], sl_[:])
                nc.vector.tensor_mul(ag[:], gsl, rg[:])
                nc.vector.tensor_mul(al[:], g1[:], rl[:])
                bg_t = bcp.tile([128, TCH], F32, tag="bcast", name=f"bg_t{r}")
                bl_t = bcp.tile([128, TCH], F32, tag="bcast", name=f"bl_t{r}")
                nc.gpsimd.partition_broadcast(bg_t[:], ag[:])
                nc.gpsimd.partition_broadcast(bl_t[:], al[:])
                t1 = combp.tile([128, TCH], F32, tag="comb", name=f"t1{r}")
                ao = aoutp.tile([128, TCH], F32, tag="aout", name=f"ao{r}")
                nc.vector.tensor_mul(t1[:], pg[:], bg_t[:])
                nc.vector.tensor_mul(pl[:], pl[:], bl_t[:])
                nc.vector.tensor_add(ao[:], t1[:], pl[:])

                # ship finished 256-col blocks to a2a staging
                for i, c in enumerate((2 * n, 2 * n + 1)):
                    nc.sync.dma_start(out=a2ai[c, h * D:(h + 1) * D, :],
                                      in_=ao[:, i * TSL:(i + 1) * TSL])

        pssm.release()
        pspv.release()
        psqk.release()
        combp.release()
        bcp.release()
        rcpp.release()
        ropet.release()
        expp.release()
        work.release()
        aoutp.release()

        # ========= phase 3: all-to-all reshard (heads -> tokens) =========
        nc.gpsimd.collective_compute(
            "AllToAll", mybir.AluOpType.bypass,
            replica_groups=[list(range(NCORES))],
            ins=[a2ai[:].opt()], outs=[a2ao[:].opt()])

        opool = tc.alloc_tile_pool(name="opool", bufs=1)
        wop = tc.alloc_tile_pool(name="wop", bufs=4)
        osb = tc.alloc_tile_pool(name="osb", bufs=4)
        pso = tc.alloc_tile_pool(name="pso", bufs=8, space="PSUM")

        afull = opool.tile([128, KT, TSL], F32R)
        nc.sync.dma_start(
            out=afull[:],
            in_=a2ao[:].rearrange("c p n -> (c p) n")
                       .rearrange("(k p) n -> p k n", p=128).bitcast(F32R))

        # ============ phase 4: o_proj for our token slice ============
        pss2 = [[pso.tile([128, TCH], F32, tag="po", name=f"po_{tt}_{e}")
                 for e in range(NCH)] for tt in range(2)]
        for k in range(KT):
            wo_t = wop.tile([128, HID], F32R, tag="wo", name=f"wo{k}")
            nc.sync.dma_start(out=wo_t[:],
                              in_=WO[k * 128:(k + 1) * 128, :].bitcast(F32R))
            for tt in range(2):
                for e in range(NCH):
                    nc.tensor.matmul(pss2[tt][e][:],
                                     afull[:, k, tt * 128:(tt + 1) * 128],
                                     wo_t[:, e * TCH:(e + 1) * TCH],
                                     start=(k == 0), stop=(k == KT - 1))
        for tt in range(2):
            for e in range(NCH):
                ot = osb.tile([128, TCH], F32, tag="ot", name=f"ot{tt}_{e}")
                nc.vector.tensor_copy(ot[:], pss2[tt][e][:])
                nc.sync.dma_start(
                    out=OUT[tt * 128:(tt + 1) * 128,
                            e * TCH:(e + 1) * TCH],
                    in_=ot[:])
        pso.release()
        osb.release()
        wop.release()
        opool.release()
        dram.release()
        const.release()

    nc.compile()
    return nc


def _host_prep(hidden_states, positions, k_global, v_global, w_qkv, w_o,
               w_gate, b_gate):
    """Layout-only host transforms + constant tables -> per-core in_maps."""
    f32 = np.float32
    hs = np.ascontiguousarray(np.asarray(hidden_states, f32))
    pos = np.asarray(positions)
    kg = np.asarray(k_global, f32)
    vg = np.asarray(v_global, f32)
    wqkv = np.asarray(w_qkv, f32)
    wo = np.ascontiguousarray(np.asarray(w_o, f32))
    wg = np.asarray(w_gate, f32)
    bg = np.asarray(b_gate, f32)

    hst = np.ascontiguousarray(hs.T)

    half = D // 2
    inv_freq = (THETA ** (-np.arange(half, dtype=f32) / half)).astype(f32)
    ang = pos.astype(f32)[:, None] * inv_freq[None, :]
    cos_t = np.cos(ang).astype(f32).T       # [64, T]
    sin_t = np.sin(ang).astype(f32).T
    csf = np.ascontiguousarray(np.concatenate([cos_t, cos_t], axis=0))
    snf = np.ascontiguousarray(np.concatenate([-sin_t, sin_t], axis=0))

    p = np.arange(128, dtype=np.int64)[:, None]
    # causal diag-band base mask: CM[j][p, x] = MASKC[p, x + (3-j)*128]
    yc = np.arange(896, dtype=np.int64)[None, :]
    maskc = np.where(yc - p - 384 >= 0, 0.0, MASKV).astype(f32)
    # local band base mask: LM[jj][p, x] = MASKL[p, x + 640 - 128*jj]
    yl = np.arange(1152, dtype=np.int64)[None, :]
    dl = yl - 512 - p
    maskl = np.where((dl >= 0) & (dl <= WIN), 0.0, MASKV).astype(f32)

    ones = np.ones((128, 1), f32)
    idn = np.eye(128, dtype=f32)

    in_maps = []
    for c in range(NCORES):
        g = c // 2
        wq = wqkv[:, 2 * c * D:(2 * c + 2) * D]
        wk = wqkv[:, HQ * D + g * D:HQ * D + (g + 1) * D]
        wv = wqkv[:, (HQ + HK) * D + g * D:(HQ + HK) * D + (g + 1) * D]
        in_maps.append({
            "HST": hst,
            "WQKV": np.ascontiguousarray(np.concatenate([wq, wk, wv], axis=1)),
            "KGT": np.ascontiguousarray(kg[:, g * D:(g + 1) * D].T),
            "VG": np.ascontiguousarray(vg[:, g * D:(g + 1) * D]),
            "WO": wo,
            "WG": np.ascontiguousarray(wg[:, 2 * c:2 * c + 2]),
            "BG": np.ascontiguousarray(bg[2 * c:2 * c + 2].reshape(1, 2)),
            "CSF": csf,
            "SNF": snf,
            "ONES": ones,
            "IDN": idn,
            "MASKC": maskc,
            "MASKL": maskl,
        })
    return in_maps


def kernel(**inputs):
    if "nc" not in _CACHE:
        _CACHE["nc"] = _build()
    nc = _CACHE["nc"]
    in_maps = _host_prep(**inputs)
    res = run_bass_kernel_spmd(nc, in_maps, core_ids=list(range(NCORES)))
    out = np.concatenate([res.results[c]["OUT"] for c in range(NCORES)], axis=0)
    return out.astype(np.float32)


# revision 15
# speedup vs baseline: 1.2677x; 1.0618x over previous
"""Trainium2 Bass kernel for LoopCoderAttention (sparse_attention).

Head-sharded tensor parallelism over 8 NeuronCores:
  core c owns query heads {2c, 2c+1} and KV head c//2.
All on-device tensors live in transposed [feature, token] layout so every
matmul contracts along the partition dim with zero on-device transposes
(except v, which needs one PE transpose per 128-tile).

o_proj: a 2MB AllToAll reshards attention output from head-sharded to
token-sharded; each core then runs the full 2048-deep contraction for its
256-token slice (the "all-reduce" happens inside the matmul accumulation).
"""
import sys
sys.path.insert(0, '/opt/trn_rl_repo')
import numpy as np
import concourse.bass as bass
import concourse.mybir as mybir
import concourse.tile as tile
from concourse import bacc
from concourse.bass_utils import run_bass_kernel_spmd

T = 2048
HID = 2048
HQ = 16
HK = 4
D = 128
WIN = 64
THETA = 10000.0
SCALE = D ** -0.5
NCORES = 8
TCH = 512                 # t-chunk (matmul free dim)
NCH = T // TCH            # 4 chunks
KT = HID // 128           # 16 k-tiles for 2048-deep contractions
ST = T // 128             # 16 s-tiles
TSL = T // NCORES         # 256-token output slice per core
MASKV = -1e9

F32 = mybir.dt.float32
F32R = mybir.dt.float32r
AF = mybir.ActivationFunctionType

_CACHE = {}


def _build():
    nc = bacc.Bacc("TRN2", target_bir_lowering=False, debug=False,
                   num_devices=NCORES)
    HST = nc.dram_tensor("HST", [HID, T], F32, kind="ExternalInput").ap()
    WQKV = nc.dram_tensor("WQKV", [HID, 512], F32, kind="ExternalInput").ap()
    KGT = nc.dram_tensor("KGT", [D, T], F32, kind="ExternalInput").ap()
    VG = nc.dram_tensor("VG", [T, D], F32, kind="ExternalInput").ap()
    WO = nc.dram_tensor("WO", [HID, HID], F32, kind="ExternalInput").ap()
    WG = nc.dram_tensor("WG", [D, 2], F32, kind="ExternalInput").ap()
    BG = nc.dram_tensor("BG", [1, 2], F32, kind="ExternalInput").ap()
    CSF = nc.dram_tensor("CSF", [128, T], F32, kind="ExternalInput").ap()
    SNF = nc.dram_tensor("SNF", [128, T], F32, kind="ExternalInput").ap()
    ONES = nc.dram_tensor("ONES", [128, 1], F32, kind="ExternalInput").ap()
    IDN = nc.dram_tensor("IDN", [128, 128], F32, kind="ExternalInput").ap()
    MASKC = nc.dram_tensor("MASKC", [128, 896], F32, kind="ExternalInput").ap()
    MASKL = nc.dram_tensor("MASKL", [128, 1152], F32, kind="ExternalInput").ap()
    OUT = nc.dram_tensor("OUT", [TSL, HID], F32, kind="ExternalOutput").ap()

    with tile.TileContext(nc) as tc:
        # pools are a strict stack: creation order is the reverse of the
        # release order at each phase boundary
        const = tc.alloc_tile_pool(name="const", bufs=1)
        dram = tc.alloc_tile_pool(name="dram", bufs=1, space="DRAM")
        aoutp = tc.alloc_tile_pool(name="aoutp", bufs=3)
        work = tc.alloc_tile_pool(name="work", bufs=1)
        expp = tc.alloc_tile_pool(name="expp", bufs=3)
        ropet = tc.alloc_tile_pool(name="ropet", bufs=3)
        rcpp = tc.alloc_tile_pool(name="rcpp", bufs=8)
        bcp = tc.alloc_tile_pool(name="bcp", bufs=3)
        combp = tc.alloc_tile_pool(name="combp", bufs=5)
        wqkvp = tc.alloc_tile_pool(name="wqkvp", bufs=1)
        chunkp = tc.alloc_tile_pool(name="chunkp", bufs=2)
        hsp = tc.alloc_tile_pool(name="hsp", bufs=6)
        ps1 = tc.alloc_tile_pool(name="ps1", bufs=6, space="PSUM")

        # ---- phase-1 constants first (critical path to first matmul) ----
        wqkv_sb = wqkvp.tile([128, KT, 512], F32R)
        wqkv_view = WQKV.rearrange("(k p) c -> p k c", p=128).bitcast(F32R)
        for k in range(KT):
            nc.sync.dma_start(out=wqkv_sb[:, k, :], in_=wqkv_view[:, k, :])
        csf_sb = const.tile([128, T], F32)
        nc.sync.dma_start(out=csf_sb[:], in_=CSF)
        snf_sb = const.tile([128, T], F32)
        nc.sync.dma_start(out=snf_sb[:], in_=SNF)
        idn_sb = const.tile([128, 128], F32)
        nc.sync.dma_start(out=idn_sb[:], in_=IDN)
        wg_sb = const.tile([D, 2], F32R)
        nc.sync.dma_start(out=wg_sb[:], in_=WG.bitcast(F32R))
        bg_sb = const.tile([1, 2], F32)
        nc.sync.dma_start(out=bg_sb[:], in_=BG)
        # attention-phase constants (scheduler fills DMA idle time)
        kgt_sb = const.tile([D, T], F32R)
        vg_sb = const.tile([128, ST, D], F32R)
        ones_sb = const.tile([128, 1], F32R)
        maskc_sb = const.tile([128, 896], F32)
        maskl_sb = const.tile([128, 1152], F32)

        # ---- persistent work tiles (through attention) ----
        qrot = work.tile([128, 2, T], F32R)
        krot = work.tile([128, T], F32R)
        vcur = work.tile([128, ST, D], F32R)   # current v in [s, d] tiles
        gate = work.tile([1, 8 * TCH], F32)    # slot 2n+h along free dim

        a2ai = dram.tile([NCORES, 2 * D, TSL], F32)
        a2ao = dram.tile([NCORES, 2 * D, TSL], F32)

        def rope_chunk(dst_full, src, n):
            """dst_full[:, n*TCH:...] = neox-rope of chunk tile src [128, TCH].

            rot = src * [cos;cos] + rot90(src) * [-sin;sin], where rot90 swaps
            the two 64-partition halves (built with two SBUF->SBUF DMAs since
            DVE ops require matching base partitions).
            """
            sl = bass.ds(n * TCH, TCH)
            sr = ropet.tile([128, TCH], F32, tag="ropesr", name=f"sr{n}")
            nc.sync.dma_start(out=sr[0:64, :], in_=src[64:128, :])
            nc.sync.dma_start(out=sr[64:128, :], in_=src[0:64, :])
            ta = ropet.tile([128, TCH], F32, tag="ropetmp", name=f"ra{n}")
            tb = ropet.tile([128, TCH], F32, tag="ropetmp", name=f"rb{n}")
            nc.vector.tensor_mul(ta[:], src[:], csf_sb[:, sl])
            nc.vector.tensor_mul(tb[:], sr[:], snf_sb[:, sl])
            nc.vector.tensor_add(dst_full[:, sl], ta[:], tb[:])

        # ================= phase 1: qkvT = wqkv^T @ hsT =================
        for n in range(NCH):
            pss = [ps1.tile([128, TCH], F32, tag="ps1t", name=f"ps1_{n}_{m}")
                   for m in range(4)]
            for k in range(KT):
                hs_t = hsp.tile([128, TCH], F32R)
                nc.sync.dma_start(
                    out=hs_t[:],
                    in_=HST[k * 128:(k + 1) * 128,
                            n * TCH:(n + 1) * TCH].bitcast(F32R))
                for m in range(4):
                    nc.tensor.matmul(pss[m][:],
                                     wqkv_sb[:, k, m * 128:(m + 1) * 128],
                                     hs_t[:],
                                     start=(k == 0), stop=(k == KT - 1))
            sl = bass.ds(n * TCH, TCH)
            q0c = chunkp.tile([128, TCH], F32, tag="q0c")
            q1c = chunkp.tile([128, TCH], F32, tag="q1c")
            kc = chunkp.tile([128, TCH], F32, tag="kc")
            vc = chunkp.tile([128, TCH], F32, tag="vc")
            nc.scalar.activation(q0c[:], pss[0][:], AF.Copy)
            nc.scalar.activation(q1c[:], pss[1][:], AF.Copy)
            nc.scalar.activation(kc[:], pss[2][:], AF.Copy)
            nc.vector.tensor_copy(vc[:], pss[3][:])

            rope_chunk(qrot[:, 0, :], q0c, n)
            rope_chunk(qrot[:, 1, :], q1c, n)
            rope_chunk(krot, kc, n)

            # transpose v tiles of this chunk: vcur[s] = vc[:, j*128:...]^T
            for j in range(4):
                s = 4 * n + j
                pt = ps1.tile([128, 128], F32, tag="ps1t", name=f"pt{s}")
                nc.tensor.transpose(pt[:], vc[:, j * 128:(j + 1) * 128],
                                    idn_sb[:])
                nc.vector.tensor_copy(vcur[:, s, :], pt[:])

            # gate for this chunk (both heads)
            for h in range(2):
                r = 2 * n + h
                gp = ps1.tile([1, TCH], F32, tag="ps1g", name=f"gp{r}", bufs=2)
                nc.tensor.matmul(gp[:], wg_sb[:, h:h + 1], qrot[:, h, sl],
                                 start=True, stop=True)
                nc.scalar.activation(gate[0:1, r * TCH:(r + 1) * TCH], gp[:],
                                     AF.Sigmoid, bias=bg_sb[0:1, h:h + 1])

        nc.sync.dma_start(out=kgt_sb[:], in_=KGT.bitcast(F32R))
        nc.sync.dma_start(out=vg_sb[:],
                          in_=VG.rearrange("(s p) d -> p s d", p=128).bitcast(F32R))
        nc.sync.dma_start(out=ones_sb[:], in_=ONES.bitcast(F32R))
        nc.sync.dma_start(out=maskc_sb[:], in_=MASKC)
        nc.sync.dma_start(out=maskl_sb[:], in_=MASKL)

        ps1.release()
        hsp.release()
        chunkp.release()
        wqkvp.release()

        psqk = tc.alloc_tile_pool(name="psqk", bufs=3, space="PSUM")
        pspv = tc.alloc_tile_pool(name="pspv", bufs=3, space="PSUM")
        pssm = tc.alloc_tile_pool(name="pssm", bufs=2, space="PSUM")

        # ============ phase 2: attention (global + local) ============
        for h in range(2):
            for n in range(NCH):
                sl = bass.ds(n * TCH, TCH)
                q_ap = qrot[:, h, sl]

                def pass_(kT_ap, v_ap, s_tiles, mask_ap_of, pfx):
                    pv = pspv.tile([128, TCH], F32, tag="pv", name=f"pv{pfx}")
                    sm = pssm.tile([1, TCH], F32, tag="sm", name=f"sm{pfx}")
                    first = True
                    for s in s_tiles:
                        qk = psqk.tile([128, TCH], F32, tag="qk",
                                       name=f"qk{pfx}_{s}")
                        nc.tensor.matmul(qk[:], kT_ap[:, s * 128:(s + 1) * 128],
                                         q_ap, start=True, stop=True)
                        m_ap = mask_ap_of(s)
                        if m_ap is not None:
                            nc.vector.tensor_add(qk[:], qk[:], m_ap)
                        ex = expp.tile([128, TCH], F32R, tag="ex", name=f"ex{pfx}_{s}")
                        nc.scalar.activation(ex[:], qk[:], AF.Exp, scale=SCALE)
                        last = (s == s_tiles[-1])
                        nc.tensor.matmul(pv[:], v_ap[:, s, :], ex[:],
                                         start=first, stop=last)
                        nc.tensor.matmul(sm[:], ones_sb[:], ex[:],
                                         start=first, stop=last)
                        first = False
                    return pv, sm

                # global pass over cached KV: causal mask on diagonal tiles
                gs = list(range(0, 4 * n + 4))

                def gmask(s, n=n):
                    j = s - 4 * n
                    if j < 0:
                        return None
                    off = (3 - j) * 128
                    return maskc_sb[:, off:off + TCH]

                pv_g, sm_g = pass_(kgt_sb, vg_sb, gs, gmask, f"g{h}{n}")

                # local pass over current KV: sliding-window band masks
                ls = [s for s in range(4 * n - 1, 4 * n + 4) if s >= 0]

                def lmask(s, n=n):
                    jj = s - (4 * n - 1)
                    off = 640 - 128 * jj
                    return maskl_sb[:, off:off + TCH]

                pv_l, sm_l = pass_(krot, vcur, ls, lmask, f"l{h}{n}")

                # drain sums fast (frees pssm); pv stays in psum until combine
                r = 2 * n + h
                sg = rcpp.tile([1, TCH], F32, tag="rcp", name=f"sg{r}")
                sl_ = rcpp.tile([1, TCH], F32, tag="rcp", name=f"sl{r}")
                nc.scalar.activation(sg[:], sm_g[:], AF.Copy)
                nc.scalar.activation(sl_[:], sm_l[:], AF.Copy)
                # a_g = gate/sum_g ; a_l = (1-gate)/sum_l
                gsl = gate[0:1, r * TCH:(r + 1) * TCH]
                ag = rcpp.tile([1, TCH], F32, tag="rcp", name=f"ag{r}")
                al = rcpp.tile([1, TCH], F32, tag="rcp", name=f"al{r}")
                g1 = rcpp.tile([1, TCH], F32, tag="rcp", name=f"g1{r}")
                nc.vector.tensor_scalar(g1[:], gsl, -1.0, 1.0,
                                        mybir.AluOpType.mult,
                                        mybir.AluOpType.add)
                rg = rcpp.tile([1, TCH], F32, tag="rcp", name=f"rg{r}")
                rl = rcpp.tile([1, TCH], F32, tag="rcp", name=f"rl{r}")
                nc.vector.reciprocal_approx_fast(rg[:], sg[:])
                nc.vector.reciprocal_approx_fast(rl[:], sl_[:])
                nc.vector.tensor_mul(ag[:], gsl, rg[:])
                nc.vector.tensor_mul(al[:], g1[:], rl[:])
                bg_t = bcp.tile([128, TCH], F32, tag="bcast", name=f"bg_t{r}")
                bl_t = bcp.tile([128, TCH], F32, tag="bcast", name=f"bl_t{r}")
                nc.gpsimd.partition_broadcast(bg_t[:], ag[:])
                nc.gpsimd.partition_broadcast(bl_t[:], al[:])
                t1 = combp.tile([128, TCH], F32, tag="comb", name=f"t1{r}")
                t2 = combp.tile([128, TCH], F32, tag="comb", name=f"t2{r}")
                ao = aoutp.tile([128, TCH], F32, tag="aout", name=f"ao{r}")
                nc.vector.tensor_mul(t1[:], pv_g[:], bg_t[:])
                nc.vector.tensor_mul(t2[:], pv_l[:], bl_t[:])
                nc.vector.tensor_add(ao[:], t1[:], t2[:])

                # ship finished 256-col blocks to a2a staging
                for i, c in enumerate((2 * n, 2 * n + 1)):
                    nc.sync.dma_start(out=a2ai[c, h * D:(h + 1) * D, :],
                                      in_=ao[:, i * TSL:(i + 1) * TSL])

        pssm.release()
        pspv.release()
        psqk.release()
        combp.release()
        bcp.release()
        rcpp.release()
        ropet.release()
        expp.release()
        work.release()
        aoutp.release()

        # ========= phase 3: all-to-all reshard (heads -> tokens) =========
        nc.gpsimd.collective_compute(
            "AllToAll", mybir.AluOpType.bypass,
            replica_groups=[list(range(NCORES))],
            ins=[a2ai[:].opt()], outs=[a2ao[:].opt()])

        opool = tc.alloc_tile_pool(name="opool", bufs=1)
        wop = tc.alloc_tile_pool(name="wop", bufs=4)
        osb = tc.alloc_tile_pool(name="osb", bufs=4)
        pso = tc.alloc_tile_pool(name="pso", bufs=8, space="PSUM")

        afull = opool.tile([128, KT, TSL], F32R)
        nc.sync.dma_start(
            out=afull[:],
            in_=a2ao[:].rearrange("c p n -> (c p) n")
                       .rearrange("(k p) n -> p k n", p=128).bitcast(F32R))

        # ============ phase 4: o_proj for our token slice ============
        pss2 = [[pso.tile([128, TCH], F32, tag="po", name=f"po_{tt}_{e}")
                 for e in range(NCH)] for tt in range(2)]
        for k in range(KT):
            wo_t = wop.tile([128, HID], F32R, tag="wo", name=f"wo{k}")
            nc.sync.dma_start(out=wo_t[:],
                              in_=WO[k * 128:(k + 1) * 128, :].bitcast(F32R))
            for tt in range(2):
                for e in range(NCH):
                    nc.tensor.matmul(pss2[tt][e][:],
                                     afull[:, k, tt * 128:(tt + 1) * 128],
                                     wo_t[:, e * TCH:(e + 1) * TCH],
                                     start=(k == 0), stop=(k == KT - 1))
        for tt in range(2):
            for e in range(NCH):
                ot = osb.tile([128, TCH], F32, tag="ot", name=f"ot{tt}_{e}")
                nc.vector.tensor_copy(ot[:], pss2[tt][e][:])
                nc.sync.dma_start(
                    out=OUT[tt * 128:(tt + 1) * 128,
                            e * TCH:(e + 1) * TCH],
                    in_=ot[:])
        pso.release()
        osb.release()
        wop.release()
        opool.release()
        dram.release()
        const.release()

    nc.compile()
    return nc


def _host_prep(hidden_states, positions, k_global, v_global, w_qkv, w_o,
               w_gate, b_gate):
    """Layout-only host transforms + constant tables -> per-core in_maps."""
    f32 = np.float32
    hs = np.ascontiguousarray(np.asarray(hidden_states, f32))
    pos = np.asarray(positions)
    kg = np.asarray(k_global, f32)
    vg = np.asarray(v_global, f32)
    wqkv = np.asarray(w_qkv, f32)
    wo = np.ascontiguousarray(np.asarray(w_o, f32))
    wg = np.asarray(w_gate, f32)
    bg = np.asarray(b_gate, f32)

    hst = np.ascontiguousarray(hs.T)

    half = D // 2
    inv_freq = (THETA ** (-np.arange(half, dtype=f32) / half)).astype(f32)
    ang = pos.astype(f32)[:, None] * inv_freq[None, :]
    cos_t = np.cos(ang).astype(f32).T       # [64, T]
    sin_t = np.sin(ang).astype(f32).T
    csf = np.ascontiguousarray(np.concatenate([cos_t, cos_t], axis=0))
    snf = np.ascontiguousarray(np.concatenate([-sin_t, sin_t], axis=0))

    p = np.arange(128, dtype=np.int64)[:, None]
    # causal diag-band base mask: CM[j][p, x] = MASKC[p, x + (3-j)*128]
    yc = np.arange(896, dtype=np.int64)[None, :]
    maskc = np.where(yc - p - 384 >= 0, 0.0, MASKV).astype(f32)
    # local band base mask: LM[jj][p, x] = MASKL[p, x + 640 - 128*jj]
    yl = np.arange(1152, dtype=np.int64)[None, :]
    dl = yl - 512 - p
    maskl = np.where((dl >= 0) & (dl <= WIN), 0.0, MASKV).astype(f32)

    ones = np.ones((128, 1), f32)
    idn = np.eye(128, dtype=f32)

    in_maps = []
    for c in range(NCORES):
        g = c // 2
        wq = wqkv[:, 2 * c * D:(2 * c + 2) * D]
        wk = wqkv[:, HQ * D + g * D:HQ * D + (g + 1) * D]
        wv = wqkv[:, (HQ + HK) * D + g * D:(HQ + HK) * D + (g + 1) * D]
        in_maps.append({
            "HST": hst,
            "WQKV": np.ascontiguousarray(np.concatenate([wq, wk, wv], axis=1)),
            "KGT": np.ascontiguousarray(kg[:, g * D:(g + 1) * D].T),
            "VG": np.ascontiguousarray(vg[:, g * D:(g + 1) * D]),
            "WO": wo,
            "WG": np.ascontiguousarray(wg[:, 2 * c:2 * c + 2]),
            "BG": np.ascontiguousarray(bg[2 * c:2 * c + 2].reshape(1, 2)),
            "CSF": csf,
            "SNF": snf,
            "ONES": ones,
            "IDN": idn,
            "MASKC": maskc,
            "MASKL": maskl,
        })
    return in_maps


def kernel(**inputs):
    if "nc" not in _CACHE:
        _CACHE["nc"] = _build()
    nc = _CACHE["nc"]
    in_maps = _host_prep(**inputs)
    res = run_bass_kernel_spmd(nc, in_maps, core_ids=list(range(NCORES)))
    out = np.concatenate([res.results[c]["OUT"] for c in range(NCORES)], axis=0)
    return out.astype(np.float32)


# revision 23
# speedup vs baseline: 1.2828x; 1.0119x over previous
"""Trainium2 Bass kernel for LoopCoderAttention (sparse_attention).

Head-sharded tensor parallelism over 8 NeuronCores:
  core c owns query heads {2c, 2c+1} and KV head c//2.
All on-device tensors live in transposed [feature, token] layout so every
matmul contracts along the partition dim with zero on-device transposes
(except v, which needs one PE transpose per 128-tile).

o_proj: a 2MB AllToAll reshards attention output from head-sharded to
token-sharded; each core then runs the full 2048-deep contraction for its
256-token slice (the "all-reduce" happens inside the matmul accumulation).
"""
import sys
sys.path.insert(0, '/opt/trn_rl_repo')
import numpy as np
import ml_dtypes
import concourse.bass as bass
import concourse.mybir as mybir
import concourse.tile as tile
from concourse import bacc
from concourse.bass_utils import run_bass_kernel_spmd

T = 2048
HID = 2048
HQ = 16
HK = 4
D = 128
WIN = 64
THETA = 10000.0
SCALE = D ** -0.5
NCORES = 8
TCH = 512                 # t-chunk (matmul free dim)
NCH = T // TCH            # 4 chunks
KT = HID // 128           # 16 k-tiles for 2048-deep contractions
ST = T // 128             # 16 s-tiles
TSL = T // NCORES         # 256-token output slice per core
MASKV = -1e9

F32 = mybir.dt.float32
F32R = mybir.dt.float32r
BF16 = mybir.dt.bfloat16
AF = mybir.ActivationFunctionType

_CACHE = {}


def _build():
    nc = bacc.Bacc("TRN2", target_bir_lowering=False, debug=False,
                   num_devices=NCORES)
    HST = nc.dram_tensor("HST", [HID, T], F32, kind="ExternalInput").ap()
    WQKV = nc.dram_tensor("WQKV", [HID, 512], F32, kind="ExternalInput").ap()
    KGT = nc.dram_tensor("KGT", [D, T], F32, kind="ExternalInput").ap()
    VG = nc.dram_tensor("VG", [T, D], F32, kind="ExternalInput").ap()
    WO = nc.dram_tensor("WO", [HID, HID], BF16, kind="ExternalInput").ap()
    WG = nc.dram_tensor("WG", [D, 2], F32, kind="ExternalInput").ap()
    BG = nc.dram_tensor("BG", [1, 2], F32, kind="ExternalInput").ap()
    CSF = nc.dram_tensor("CSF", [128, T], F32, kind="ExternalInput").ap()
    SNF = nc.dram_tensor("SNF", [128, T], F32, kind="ExternalInput").ap()
    ONES = nc.dram_tensor("ONES", [128, 1], F32, kind="ExternalInput").ap()
    IDN = nc.dram_tensor("IDN", [128, 128], F32, kind="ExternalInput").ap()
    MASKC = nc.dram_tensor("MASKC", [128, 896], F32, kind="ExternalInput").ap()
    MASKL = nc.dram_tensor("MASKL", [128, 1152], F32, kind="ExternalInput").ap()
    OUT = nc.dram_tensor("OUT", [TSL, HID], F32, kind="ExternalOutput").ap()

    with tile.TileContext(nc) as tc:
        # pools are a strict stack: creation order is the reverse of the
        # release order at each phase boundary
        const = tc.alloc_tile_pool(name="const", bufs=1)
        dram = tc.alloc_tile_pool(name="dram", bufs=1, space="DRAM")
        aoutp = tc.alloc_tile_pool(name="aoutp", bufs=3)
        opool = tc.alloc_tile_pool(name="opool", bufs=1)
        wop = tc.alloc_tile_pool(name="wop", bufs=4)
        osb = tc.alloc_tile_pool(name="osb", bufs=2)
        work = tc.alloc_tile_pool(name="work", bufs=1)
        ropet = tc.alloc_tile_pool(name="ropet", bufs=2)
        rcpp = tc.alloc_tile_pool(name="rcpp", bufs=8)
        bcp = tc.alloc_tile_pool(name="bcp", bufs=3)
        combp = tc.alloc_tile_pool(name="combp", bufs=4)
        wqkvp = tc.alloc_tile_pool(name="wqkvp", bufs=1)
        chunkp = tc.alloc_tile_pool(name="chunkp", bufs=2)
        hsp = tc.alloc_tile_pool(name="hsp", bufs=6)
        ps1 = tc.alloc_tile_pool(name="ps1", bufs=6, space="PSUM")

        # ---- phase-1 constants first (critical path to first matmul) ----
        wqkv_sb = wqkvp.tile([128, KT, 512], F32R)
        wqkv_view = WQKV.rearrange("(k p) c -> p k c", p=128).bitcast(F32R)
        for k in range(KT):
            nc.sync.dma_start(out=wqkv_sb[:, k, :], in_=wqkv_view[:, k, :])
        csf_sb = wqkvp.tile([128, T], F32)
        nc.sync.dma_start(out=csf_sb[:], in_=CSF)
        snf_sb = wqkvp.tile([128, T], F32)
        nc.sync.dma_start(out=snf_sb[:], in_=SNF)
        idn_sb = wqkvp.tile([128, 128], F32)
        nc.sync.dma_start(out=idn_sb[:], in_=IDN)
        wg_sb = const.tile([D, 2], F32R)
        nc.sync.dma_start(out=wg_sb[:], in_=WG.bitcast(F32R))
        bg_sb = const.tile([1, 2], F32)
        nc.sync.dma_start(out=bg_sb[:], in_=BG)
        # attention-phase constants (scheduler fills DMA idle time)
        kgt_sb = const.tile([D, T], F32R)
        vg_sb = const.tile([128, ST, D], F32R)
        ones_sb = const.tile([128, 1], F32R)
        maskc_sb = const.tile([128, 896], F32)
        maskl_sb = const.tile([128, 1152], F32)

        # ---- persistent work tiles (through attention) ----
        qrot = work.tile([128, 2, T], F32R)
        krot = work.tile([128, T], F32R)
        vcur = work.tile([128, ST, D], F32R)   # current v in [s, d] tiles
        gate = work.tile([8, TCH], F32)        # row 2n+h (DMA-staged access)

        a2ai_hi = dram.tile([NCORES, 2 * D, TSL // 2], BF16)
        a2ao_hi = dram.tile([NCORES, 2 * D, TSL // 2], BF16)
        a2ai_lo = dram.tile([NCORES, 2 * D, TSL // 2], BF16)
        a2ao_lo = dram.tile([NCORES, 2 * D, TSL // 2], BF16)

        def rope_chunk(dst_full, src, n):
            """dst_full[:, n*TCH:...] = neox-rope of chunk tile src [128, TCH].

            rot = src * [cos;cos] + rot90(src) * [-sin;sin], where rot90 swaps
            the two 64-partition halves (built with two SBUF->SBUF DMAs since
            DVE ops require matching base partitions).
            """
            sl = bass.ds(n * TCH, TCH)
            sr = ropet.tile([128, TCH], F32, tag="ropesr", name=f"sr{n}")
            nc.sync.dma_start(out=sr[0:64, :], in_=src[64:128, :])
            nc.sync.dma_start(out=sr[64:128, :], in_=src[0:64, :])
            ta = ropet.tile([128, TCH], F32, tag="ropetmp", name=f"ra{n}")
            tb = ropet.tile([128, TCH], F32, tag="ropetmp", name=f"rb{n}")
            nc.vector.tensor_mul(ta[:], src[:], csf_sb[:, sl])
            nc.vector.tensor_mul(tb[:], sr[:], snf_sb[:, sl])
            nc.vector.tensor_add(dst_full[:, sl], ta[:], tb[:])

        # ================= phase 1: qkvT = wqkv^T @ hsT =================
        for n in reversed(range(NCH)):
            pss = [ps1.tile([128, TCH], F32, tag="ps1t", name=f"ps1_{n}_{m}")
                   for m in range(4)]
            for k in range(KT):
                hs_t = hsp.tile([128, TCH], F32R)
                nc.sync.dma_start(
                    out=hs_t[:],
                    in_=HST[k * 128:(k + 1) * 128,
                            n * TCH:(n + 1) * TCH].bitcast(F32R))
                for m in range(4):
                    nc.tensor.matmul(pss[m][:],
                                     wqkv_sb[:, k, m * 128:(m + 1) * 128],
                                     hs_t[:],
                                     start=(k == 0), stop=(k == KT - 1))
            sl = bass.ds(n * TCH, TCH)
            q0c = chunkp.tile([128, TCH], F32, tag="q0c")
            q1c = chunkp.tile([128, TCH], F32, tag="q1c")
            kc = chunkp.tile([128, TCH], F32, tag="kc")
            vc = chunkp.tile([128, TCH], F32, tag="vc")
            nc.scalar.activation(q0c[:], pss[0][:], AF.Copy)
            nc.scalar.activation(q1c[:], pss[1][:], AF.Copy)
            nc.scalar.activation(kc[:], pss[2][:], AF.Copy)
            nc.vector.tensor_copy(vc[:], pss[3][:])

            rope_chunk(qrot[:, 0, :], q0c, n)
            rope_chunk(qrot[:, 1, :], q1c, n)
            rope_chunk(krot, kc, n)

            # transpose v tiles of this chunk: vcur[s] = vc[:, j*128:...]^T
            for j in range(4):
                s = 4 * n + j
                pt = ps1.tile([128, 128], F32, tag="ps1t", name=f"pt{s}")
                nc.tensor.transpose(pt[:], vc[:, j * 128:(j + 1) * 128],
                                    idn_sb[:])
                nc.vector.tensor_copy(vcur[:, s, :], pt[:])

            # gate for this chunk (both heads)
            for h in range(2):
                r = 2 * n + h
                gp = ps1.tile([1, TCH], F32, tag="ps1g", name=f"gp{r}", bufs=2)
                nc.tensor.matmul(gp[:], wg_sb[:, h:h + 1], qrot[:, h, sl],
                                 start=True, stop=True)
                gst = chunkp.tile([1, TCH], F32, tag="gst", name=f"gst{r}")
                nc.scalar.activation(gst[:], gp[:], AF.Sigmoid,
                                     bias=bg_sb[0:1, h:h + 1])
                nc.sync.dma_start(out=gate[r:r + 1, :], in_=gst[:])

        nc.sync.dma_start(out=kgt_sb[:], in_=KGT.bitcast(F32R))
        nc.sync.dma_start(out=vg_sb[:],
                          in_=VG.rearrange("(s p) d -> p s d", p=128).bitcast(F32R))
        nc.sync.dma_start(out=ones_sb[:], in_=ONES.bitcast(F32R))
        nc.sync.dma_start(out=maskc_sb[:], in_=MASKC)
        nc.sync.dma_start(out=maskl_sb[:], in_=MASKL)

        ps1.release()
        hsp.release()
        chunkp.release()
        wqkvp.release()

        afull_hi = opool.tile([128, KT, TSL // 2], BF16)
        afull_lo = opool.tile([128, KT, TSL // 2], BF16)

        expp = tc.alloc_tile_pool(name="expp", bufs=6)
        psqk = tc.alloc_tile_pool(name="psqk", bufs=3, space="PSUM")
        pspv = tc.alloc_tile_pool(name="pspv", bufs=3, space="PSUM")
        pssm = tc.alloc_tile_pool(name="pssm", bufs=2, space="PSUM")

        # ============ phase 2: attention (global + local) ============
        # chunks descend so the high-token half finishes first and its
        # all-to-all overlaps the low-token half's compute
        for n in reversed(range(NCH)):
            for h in range(2):
                sl = bass.ds(n * TCH, TCH)
                q_ap = qrot[:, h, sl]

                def pass_(kT_ap, v_ap, s_tiles, mask_ap_of, pfx):
                    pv = pspv.tile([128, TCH], F32, tag="pv", name=f"pv{pfx}")
                    sm = pssm.tile([1, TCH], F32, tag="sm", name=f"sm{pfx}")
                    first = True
                    for s in s_tiles:
                        qk = psqk.tile([128, TCH], F32, tag="qk",
                                       name=f"qk{pfx}_{s}")
                        nc.tensor.matmul(qk[:], kT_ap[:, s * 128:(s + 1) * 128],
                                         q_ap, start=True, stop=True)
                        m_ap = mask_ap_of(s)
                        if m_ap is not None:
                            nc.vector.tensor_add(qk[:], qk[:], m_ap)
                        ex = expp.tile([128, TCH], F32R, tag="ex", name=f"ex{pfx}_{s}")
                        nc.scalar.activation(ex[:], qk[:], AF.Exp, scale=SCALE)
                        last = (s == s_tiles[-1])
                        nc.tensor.matmul(pv[:], v_ap[:, s, :], ex[:],
                                         start=first, stop=last)
                        nc.tensor.matmul(sm[:], ones_sb[:], ex[:],
                                         start=first, stop=last)
                        first = False
                    return pv, sm

                # global pass over cached KV: causal mask on diagonal tiles
                gs = list(range(0, 4 * n + 4))

                def gmask(s, n=n):
                    j = s - 4 * n
                    if j < 0:
                        return None
                    off = (3 - j) * 128
                    return maskc_sb[:, off:off + TCH]

                pv_g, sm_g = pass_(kgt_sb, vg_sb, gs, gmask, f"g{h}{n}")

                # local pass over current KV: sliding-window band masks
                ls = [s for s in range(4 * n - 1, 4 * n + 4) if s >= 0]

                def lmask(s, n=n):
                    jj = s - (4 * n - 1)
                    off = 640 - 128 * jj
                    return maskl_sb[:, off:off + TCH]

                pv_l, sm_l = pass_(krot, vcur, ls, lmask, f"l{h}{n}")

                # drain sums fast (frees pssm); pv stays in psum until combine
                r = 2 * n + h
                sg = rcpp.tile([1, TCH], F32, tag="rcp", name=f"sg{r}")
                sl_ = rcpp.tile([1, TCH], F32, tag="rcp", name=f"sl{r}")
                nc.scalar.activation(sg[:], sm_g[:], AF.Copy)
                nc.scalar.activation(sl_[:], sm_l[:], AF.Copy)
                # a_g = gate/sum_g ; a_l = (1-gate)/sum_l
                gsl_t = rcpp.tile([1, TCH], F32, tag="rcp", name=f"gsl{r}")
                nc.sync.dma_start(out=gsl_t[:], in_=gate[r:r + 1, :])
                gsl = gsl_t[:]
                ag = rcpp.tile([1, TCH], F32, tag="rcp", name=f"ag{r}")
                al = rcpp.tile([1, TCH], F32, tag="rcp", name=f"al{r}")
                g1 = rcpp.tile([1, TCH], F32, tag="rcp", name=f"g1{r}")
                nc.vector.tensor_scalar(g1[:], gsl, -1.0, 1.0,
                                        mybir.AluOpType.mult,
                                        mybir.AluOpType.add)
                rg = rcpp.tile([1, TCH], F32, tag="rcp", name=f"rg{r}")
                rl = rcpp.tile([1, TCH], F32, tag="rcp", name=f"rl{r}")
                nc.vector.reciprocal_approx_fast(rg[:], sg[:])
                nc.vector.reciprocal_approx_fast(rl[:], sl_[:])
                nc.vector.tensor_mul(ag[:], gsl, rg[:])
                nc.vector.tensor_mul(al[:], g1[:], rl[:])
                bg_t = bcp.tile([128, TCH], F32, tag="bcast", name=f"bg_t{r}")
                bl_t = bcp.tile([128, TCH], F32, tag="bcast", name=f"bl_t{r}")
                nc.gpsimd.partition_broadcast(bg_t[:], ag[:])
                nc.gpsimd.partition_broadcast(bl_t[:], al[:])
                t1 = combp.tile([128, TCH], F32, tag="comb", name=f"t1{r}")
                t2 = combp.tile([128, TCH], F32, tag="comb", name=f"t2{r}")
                ao = aoutp.tile([128, TCH], BF16, tag="aout", name=f"ao{r}")
                nc.vector.tensor_mul(t1[:], pv_g[:], bg_t[:])
                nc.vector.tensor_mul(t2[:], pv_l[:], bl_t[:])
                nc.vector.tensor_add(ao[:], t1[:], t2[:])

                # ship finished 128-col blocks to a2a staging
                # token 1024+128c (hi) / 128c (lo) lives in chunk n at column
                # offset 128j; each unit covers 4 destination quarter-blocks
                buf = a2ai_hi if n >= 2 else a2ai_lo
                c0 = (n - 2) * 4 if n >= 2 else n * 4
                for j in range(4):
                    nc.sync.dma_start(
                        out=buf[c0 + j, h * D:(h + 1) * D, :],
                        in_=ao[:, j * 128:(j + 1) * 128])

                if n == 2 and h == 1:
                    # all-to-all #1: high-token halves (overlaps chunks 1,0)
                    nc.gpsimd.collective_compute(
                        "AllToAll", mybir.AluOpType.bypass,
                        replica_groups=[list(range(NCORES))],
                        ins=[a2ai_hi[:].opt()], outs=[a2ao_hi[:].opt()])
                    nc.sync.dma_start(
                        out=afull_hi[:],
                        in_=a2ao_hi[:].rearrange("c p n -> (c p) n")
                            .rearrange("(k p) n -> p k n", p=128))

        pssm.release()
        pspv.release()
        psqk.release()
        expp.release()
        combp.release()
        bcp.release()
        rcpp.release()
        ropet.release()
        work.release()

        # ========= phase 3: all-to-all #2 (low-token halves) =========
        nc.gpsimd.collective_compute(
            "AllToAll", mybir.AluOpType.bypass,
            replica_groups=[list(range(NCORES))],
            ins=[a2ai_lo[:].opt()], outs=[a2ao_lo[:].opt()])

        pso = tc.alloc_tile_pool(name="pso", bufs=8, space="PSUM")

        nc.sync.dma_start(
            out=afull_lo[:],
            in_=a2ao_lo[:].rearrange("c p n -> (c p) n")
                .rearrange("(k p) n -> p k n", p=128))

        # ============ phase 4: o_proj for our token slice ============
        # OUT rows 0-127 = low half-slice, rows 128-255 = high half-slice
        afulls = [afull_lo, afull_hi]
        pss2 = [[pso.tile([128, TCH], F32, tag="po", name=f"po_{tt}_{e}")
                 for e in range(NCH)] for tt in range(2)]
        for k in range(KT):
            wo_t = wop.tile([128, HID], BF16, tag="wo", name=f"wo{k}")
            nc.sync.dma_start(out=wo_t[:],
                              in_=WO[k * 128:(k + 1) * 128, :])
            for tt in range(2):
                for e in range(NCH):
                    nc.tensor.matmul(pss2[tt][e][:],
                                     afulls[tt][:, k, :],
                                     wo_t[:, e * TCH:(e + 1) * TCH],
                                     start=(k == 0), stop=(k == KT - 1))
        for tt in range(2):
            for e in range(NCH):
                ot = osb.tile([128, TCH], F32, tag="ot", name=f"ot{tt}_{e}")
                nc.vector.tensor_copy(ot[:], pss2[tt][e][:])
                nc.sync.dma_start(
                    out=OUT[tt * 128:(tt + 1) * 128,
                            e * TCH:(e + 1) * TCH],
                    in_=ot[:])
        pso.release()
        osb.release()
        wop.release()
        opool.release()
        aoutp.release()
        dram.release()
        const.release()

    nc.compile()
    return nc


def _host_prep(hidden_states, positions, k_global, v_global, w_qkv, w_o,
               w_gate, b_gate):
    """Layout-only host transforms + constant tables -> per-core in_maps."""
    f32 = np.float32
    hs = np.ascontiguousarray(np.asarray(hidden_states, f32))
    pos = np.asarray(positions)
    kg = np.asarray(k_global, f32)
    vg = np.asarray(v_global, f32)
    wqkv = np.asarray(w_qkv, f32)
    wo = np.ascontiguousarray(np.asarray(w_o, f32).astype(ml_dtypes.bfloat16))
    wg = np.asarray(w_gate, f32)
    bg = np.asarray(b_gate, f32)

    hst = np.ascontiguousarray(hs.T)

    half = D // 2
    inv_freq = (THETA ** (-np.arange(half, dtype=f32) / half)).astype(f32)
    ang = pos.astype(f32)[:, None] * inv_freq[None, :]
    cos_t = np.cos(ang).astype(f32).T       # [64, T]
    sin_t = np.sin(ang).astype(f32).T
    csf = np.ascontiguousarray(np.concatenate([cos_t, cos_t], axis=0))
    snf = np.ascontiguousarray(np.concatenate([-sin_t, sin_t], axis=0))

    p = np.arange(128, dtype=np.int64)[:, None]
    # causal diag-band base mask: CM[j][p, x] = MASKC[p, x + (3-j)*128]
    yc = np.arange(896, dtype=np.int64)[None, :]
    maskc = np.where(yc - p - 384 >= 0, 0.0, MASKV).astype(f32)
    # local band base mask: LM[jj][p, x] = MASKL[p, x + 640 - 128*jj]
    yl = np.arange(1152, dtype=np.int64)[None, :]
    dl = yl - 512 - p
    maskl = np.where((dl >= 0) & (dl <= WIN), 0.0, MASKV).astype(f32)

    ones = np.ones((128, 1), f32)
    idn = np.eye(128, dtype=f32)

    in_maps = []
    for c in range(NCORES):
        g = c // 2
        wq = wqkv[:, 2 * c * D:(2 * c + 2) * D]
        wk = wqkv[:, HQ * D + g * D:HQ * D + (g + 1) * D]
        wv = wqkv[:, (HQ + HK) * D + g * D:(HQ + HK) * D + (g + 1) * D]
        in_maps.append({
            "HST": hst,
            "WQKV": np.ascontiguousarray(np.concatenate([wq, wk, wv], axis=1)),
            "KGT": np.ascontiguousarray(kg[:, g * D:(g + 1) * D].T),
            "VG": np.ascontiguousarray(vg[:, g * D:(g + 1) * D]),
            "WO": wo,
            "WG": np.ascontiguousarray(wg[:, 2 * c:2 * c + 2]),
            "BG": np.ascontiguousarray(bg[2 * c:2 * c + 2].reshape(1, 2)),
            "CSF": csf,
            "SNF": snf,
            "ONES": ones,
            "IDN": idn,
            "MASKC": maskc,
            "MASKL": maskl,
        })
    return in_maps


def kernel(**inputs):
    if "nc" not in _CACHE:
        _CACHE["nc"] = _build()
    nc = _CACHE["nc"]
    in_maps = _host_prep(**inputs)
    res = run_bass_kernel_spmd(nc, in_maps, core_ids=list(range(NCORES)))
    out = np.empty((T, HID), np.float32)
    for c in range(NCORES):
        o = res.results[c]["OUT"]
        out[128 * c:128 * (c + 1)] = o[0:128]
        out[1024 + 128 * c:1024 + 128 * (c + 1)] = o[128:256]
    return out


# revision 26
# speedup vs baseline: 1.3581x; 1.0587x over previous
"""Trainium2 Bass kernel for LoopCoderAttention (sparse_attention).

Head-sharded tensor parallelism over 8 NeuronCores:
  core c owns query heads {2c, 2c+1} and KV head c//2.
All on-device tensors live in transposed [feature, token] layout so every
matmul contracts along the partition dim with zero on-device transposes
(except v, which needs one PE transpose per 128-tile).

o_proj: a 2MB AllToAll reshards attention output from head-sharded to
token-sharded; each core then runs the full 2048-deep contraction for its
256-token slice (the "all-reduce" happens inside the matmul accumulation).
"""
import sys
sys.path.insert(0, '/opt/trn_rl_repo')
import numpy as np
import ml_dtypes
import concourse.bass as bass
import concourse.mybir as mybir
import concourse.tile as tile
from concourse import bacc
from concourse.bass_utils import run_bass_kernel_spmd

T = 2048
HID = 2048
HQ = 16
HK = 4
D = 128
WIN = 64
THETA = 10000.0
SCALE = D ** -0.5
NCORES = 8
TCH = 512                 # t-chunk (matmul free dim)
NCH = T // TCH            # 4 chunks
KT = HID // 128           # 16 k-tiles for 2048-deep contractions
ST = T // 128             # 16 s-tiles
TSL = T // NCORES         # 256-token output slice per core
MASKV = -1e9

F32 = mybir.dt.float32
F32R = mybir.dt.float32r
BF16 = mybir.dt.bfloat16
AF = mybir.ActivationFunctionType

_CACHE = {}


def _build():
    nc = bacc.Bacc("TRN2", target_bir_lowering=False, debug=False,
                   num_devices=NCORES)
    HST = nc.dram_tensor("HST", [HID, T], F32, kind="ExternalInput").ap()
    WQKV = nc.dram_tensor("WQKV", [HID, 512], F32, kind="ExternalInput").ap()
    KGT = nc.dram_tensor("KGT", [D, T], F32, kind="ExternalInput").ap()
    VG = nc.dram_tensor("VG", [T, D], F32, kind="ExternalInput").ap()
    WO = nc.dram_tensor("WO", [HID, HID], BF16, kind="ExternalInput").ap()
    WG = nc.dram_tensor("WG", [D, 2], F32, kind="ExternalInput").ap()
    BG = nc.dram_tensor("BG", [1, 2], F32, kind="ExternalInput").ap()
    CSF = nc.dram_tensor("CSF", [128, T], F32, kind="ExternalInput").ap()
    SNF = nc.dram_tensor("SNF", [128, T], F32, kind="ExternalInput").ap()
    ONES = nc.dram_tensor("ONES", [128, 1], F32, kind="ExternalInput").ap()
    IDN = nc.dram_tensor("IDN", [128, 128], F32, kind="ExternalInput").ap()
    MASKC = nc.dram_tensor("MASKC", [128, 896], F32, kind="ExternalInput").ap()
    MASKL = nc.dram_tensor("MASKL", [128, 1152], F32, kind="ExternalInput").ap()
    OUT = nc.dram_tensor("OUT", [TSL, HID], F32, kind="ExternalOutput").ap()

    with tile.TileContext(nc) as tc:
        # pools are a strict stack: creation order is the reverse of the
        # release order at each phase boundary
        const = tc.alloc_tile_pool(name="const", bufs=1)
        dram = tc.alloc_tile_pool(name="dram", bufs=1, space="DRAM")
        aoutp = tc.alloc_tile_pool(name="aoutp", bufs=3)
        opool = tc.alloc_tile_pool(name="opool", bufs=1)
        wop = tc.alloc_tile_pool(name="wop", bufs=4)
        osb = tc.alloc_tile_pool(name="osb", bufs=2)
        work = tc.alloc_tile_pool(name="work", bufs=1)
        ropet = tc.alloc_tile_pool(name="ropet", bufs=2)
        rcpp = tc.alloc_tile_pool(name="rcpp", bufs=8)
        bcp = tc.alloc_tile_pool(name="bcp", bufs=3)
        combp = tc.alloc_tile_pool(name="combp", bufs=4)
        wqkvp = tc.alloc_tile_pool(name="wqkvp", bufs=1)
        chunkp = tc.alloc_tile_pool(name="chunkp", bufs=2)
        hsp = tc.alloc_tile_pool(name="hsp", bufs=6)
        ps1 = tc.alloc_tile_pool(name="ps1", bufs=7, space="PSUM")

        # ---- phase-1 constants first (critical path to first matmul) ----
        wqkv_sb = wqkvp.tile([128, KT, 512], F32R)
        wqkv_view = WQKV.rearrange("(k p) c -> p k c", p=128).bitcast(F32R)
        for k in range(KT):
            nc.sync.dma_start(out=wqkv_sb[:, k, :], in_=wqkv_view[:, k, :])
        csf_sb = wqkvp.tile([128, T], F32)
        snf_sb = wqkvp.tile([128, T], F32)
        idn_sb = wqkvp.tile([128, 128], F32)
        wg_sb = const.tile([D, 2], F32R)
        nc.sync.dma_start(out=wg_sb[:], in_=WG.bitcast(F32R))
        bg_sb = const.tile([1, 2], F32)
        nc.sync.dma_start(out=bg_sb[:], in_=BG)
        # attention-phase constants (scheduler fills DMA idle time)
        kgt_sb = const.tile([D, T], F32R)
        vg_sb = const.tile([128, ST, D], F32R)
        ones_sb = const.tile([128, 1], F32R)
        maskc_sb = const.tile([128, 896], F32)
        maskl_sb = const.tile([128, 1152], F32)

        # ---- persistent work tiles (through attention) ----
        qrot = work.tile([128, 2, T], F32R)
        krot = work.tile([128, T], F32R)
        vcur = work.tile([128, ST, D], F32R)   # current v in [s, d] tiles
        gate = work.tile([8, TCH], F32)        # row 2n+h (DMA-staged access)

        a2ai_hi = dram.tile([NCORES, 2 * D, TSL // 2], BF16)
        a2ao_hi = dram.tile([NCORES, 2 * D, TSL // 2], BF16)
        a2ai_lo = dram.tile([NCORES, 2 * D, TSL // 2], BF16)
        a2ao_lo = dram.tile([NCORES, 2 * D, TSL // 2], BF16)

        def rope_chunk(dst_full, src, n):
            """dst_full[:, n*TCH:...] = neox-rope of chunk tile src [128, TCH].

            rot = src * [cos;cos] + rot90(src) * [-sin;sin], where rot90 swaps
            the two 64-partition halves (built with two SBUF->SBUF DMAs since
            DVE ops require matching base partitions).
            """
            sl = bass.ds(n * TCH, TCH)
            sr = ropet.tile([128, TCH], F32, tag="ropesr", name=f"sr{n}")
            nc.sync.dma_start(out=sr[0:64, :], in_=src[64:128, :])
            nc.sync.dma_start(out=sr[64:128, :], in_=src[0:64, :])
            ta = ropet.tile([128, TCH], F32, tag="ropetmp", name=f"ra{n}")
            tb = ropet.tile([128, TCH], F32, tag="ropetmp", name=f"rb{n}")
            nc.vector.tensor_mul(ta[:], src[:], csf_sb[:, sl])
            nc.vector.tensor_mul(tb[:], sr[:], snf_sb[:, sl])
            nc.vector.tensor_add(dst_full[:, sl], ta[:], tb[:])

        nc.sync.dma_start(out=csf_sb[:], in_=CSF)
        nc.sync.dma_start(out=snf_sb[:], in_=SNF)
        nc.sync.dma_start(out=idn_sb[:], in_=IDN)

        # ================= phase 1: qkvT = wqkv^T @ hsT =================
        for n in reversed(range(NCH)):
            pss = [ps1.tile([128, TCH], F32, tag="ps1t", name=f"ps1_{n}_{m}")
                   for m in range(4)]
            for k in range(KT):
                hs_t = hsp.tile([128, TCH], F32R)
                nc.sync.dma_start(
                    out=hs_t[:],
                    in_=HST[k * 128:(k + 1) * 128,
                            n * TCH:(n + 1) * TCH].bitcast(F32R))
                for m in range(4):
                    nc.tensor.matmul(pss[m][:],
                                     wqkv_sb[:, k, m * 128:(m + 1) * 128],
                                     hs_t[:],
                                     start=(k == 0), stop=(k == KT - 1))
            sl = bass.ds(n * TCH, TCH)
            q0c = chunkp.tile([128, TCH], F32, tag="q0c")
            q1c = chunkp.tile([128, TCH], F32, tag="q1c")
            kc = chunkp.tile([128, TCH], F32, tag="kc")
            vc = chunkp.tile([128, TCH], F32, tag="vc")
            nc.scalar.activation(q0c[:], pss[0][:], AF.Copy)
            nc.scalar.activation(q1c[:], pss[1][:], AF.Copy)
            nc.scalar.activation(kc[:], pss[2][:], AF.Copy)
            nc.vector.tensor_copy(vc[:], pss[3][:])

            rope_chunk(qrot[:, 0, :], q0c, n)
            rope_chunk(qrot[:, 1, :], q1c, n)
            rope_chunk(krot, kc, n)

            # transpose v tiles of this chunk: vcur[s] = vc[:, j*128:...]^T
            for j in range(4):
                s = 4 * n + j
                pt = ps1.tile([128, 128], F32, tag="ps1g", name=f"pt{s}", bufs=1)
                nc.tensor.transpose(pt[:], vc[:, j * 128:(j + 1) * 128],
                                    idn_sb[:])
                nc.vector.tensor_copy(vcur[:, s, :], pt[:])

            # gate for this chunk (both heads)
            for h in range(2):
                r = 2 * n + h
                gp = ps1.tile([1, TCH], F32, tag="ps1g", name=f"gp{r}", bufs=1)
                nc.tensor.matmul(gp[:], wg_sb[:, h:h + 1], qrot[:, h, sl],
                                 start=True, stop=True)
                gst = chunkp.tile([1, TCH], F32, tag="gst", name=f"gst{r}")
                nc.scalar.activation(gst[:], gp[:], AF.Sigmoid,
                                     bias=bg_sb[0:1, h:h + 1])
                nc.sync.dma_start(out=gate[r:r + 1, :], in_=gst[:])

        nc.sync.dma_start(out=kgt_sb[:], in_=KGT.bitcast(F32R))
        nc.sync.dma_start(out=vg_sb[:],
                          in_=VG.rearrange("(s p) d -> p s d", p=128).bitcast(F32R))
        nc.sync.dma_start(out=ones_sb[:], in_=ONES.bitcast(F32R))
        nc.sync.dma_start(out=maskc_sb[:], in_=MASKC)
        nc.sync.dma_start(out=maskl_sb[:], in_=MASKL)

        ps1.release()
        hsp.release()
        chunkp.release()
        wqkvp.release()

        afull_hi = opool.tile([128, KT, TSL // 2], BF16)
        afull_lo = opool.tile([128, KT, TSL // 2], BF16)

        expp = tc.alloc_tile_pool(name="expp", bufs=6)
        psqk = tc.alloc_tile_pool(name="psqk", bufs=4, space="PSUM")
        pspv = tc.alloc_tile_pool(name="pspv", bufs=3, space="PSUM")
        pssm = tc.alloc_tile_pool(name="pssm", bufs=1, space="PSUM")

        # ============ phase 2: attention (global + local) ============
        # chunks descend so the high-token half finishes first and its
        # all-to-all overlaps the low-token half's compute
        for n in reversed(range(NCH)):
            for h in range(2):
                sl = bass.ds(n * TCH, TCH)
                q_ap = qrot[:, h, sl]

                def pass_(kT_ap, v_ap, s_tiles, mask_ap_of, pfx):
                    pv = pspv.tile([128, TCH], F32, tag="pv", name=f"pv{pfx}")
                    sm = pssm.tile([1, TCH], F32, tag="sm", name=f"sm{pfx}")
                    first = True
                    for s in s_tiles:
                        qk = psqk.tile([128, TCH], F32, tag="qk",
                                       name=f"qk{pfx}_{s}")
                        nc.tensor.matmul(qk[:], kT_ap[:, s * 128:(s + 1) * 128],
                                         q_ap, start=True, stop=True)
                        m_ap = mask_ap_of(s)
                        if m_ap is not None:
                            nc.vector.tensor_add(qk[:], qk[:], m_ap)
                        ex = expp.tile([128, TCH], F32R, tag="ex", name=f"ex{pfx}_{s}")
                        nc.scalar.activation(ex[:], qk[:], AF.Exp, scale=SCALE)
                        last = (s == s_tiles[-1])
                        nc.tensor.matmul(pv[:], v_ap[:, s, :], ex[:],
                                         start=first, stop=last)
                        nc.tensor.matmul(sm[:], ones_sb[:], ex[:],
                                         start=first, stop=last)
                        first = False
                    return pv, sm

                # global pass over cached KV: causal mask on diagonal tiles
                gs = list(range(0, 4 * n + 4))

                def gmask(s, n=n):
                    j = s - 4 * n
                    if j < 0:
                        return None
                    off = (3 - j) * 128
                    return maskc_sb[:, off:off + TCH]

                pv_g, sm_g = pass_(kgt_sb, vg_sb, gs, gmask, f"g{h}{n}")

                # local pass over current KV: sliding-window band masks
                ls = [s for s in range(4 * n - 1, 4 * n + 4) if s >= 0]

                def lmask(s, n=n):
                    jj = s - (4 * n - 1)
                    off = 640 - 128 * jj
                    return maskl_sb[:, off:off + TCH]

                pv_l, sm_l = pass_(krot, vcur, ls, lmask, f"l{h}{n}")

                # drain sums fast (frees pssm); pv stays in psum until combine
                r = 2 * n + h
                sg = rcpp.tile([1, TCH], F32, tag="rcp", name=f"sg{r}")
                sl_ = rcpp.tile([1, TCH], F32, tag="rcp", name=f"sl{r}")
                nc.scalar.activation(sg[:], sm_g[:], AF.Copy)
                nc.scalar.activation(sl_[:], sm_l[:], AF.Copy)
                # a_g = gate/sum_g ; a_l = (1-gate)/sum_l
                gsl_t = rcpp.tile([1, TCH], F32, tag="rcp", name=f"gsl{r}")
                nc.sync.dma_start(out=gsl_t[:], in_=gate[r:r + 1, :])
                gsl = gsl_t[:]
                ag = rcpp.tile([1, TCH], F32, tag="rcp", name=f"ag{r}")
                al = rcpp.tile([1, TCH], F32, tag="rcp", name=f"al{r}")
                g1 = rcpp.tile([1, TCH], F32, tag="rcp", name=f"g1{r}")
                nc.vector.tensor_scalar(g1[:], gsl, -1.0, 1.0,
                                        mybir.AluOpType.mult,
                                        mybir.AluOpType.add)
                rg = rcpp.tile([1, TCH], F32, tag="rcp", name=f"rg{r}")
                rl = rcpp.tile([1, TCH], F32, tag="rcp", name=f"rl{r}")
                nc.vector.reciprocal_approx_fast(rg[:], sg[:])
                nc.vector.reciprocal_approx_fast(rl[:], sl_[:])
                nc.vector.tensor_mul(ag[:], gsl, rg[:])
                nc.vector.tensor_mul(al[:], g1[:], rl[:])
                bg_t = bcp.tile([128, TCH], F32, tag="bcast", name=f"bg_t{r}")
                bl_t = bcp.tile([128, TCH], F32, tag="bcast", name=f"bl_t{r}")
                nc.gpsimd.partition_broadcast(bg_t[:], ag[:])
                nc.gpsimd.partition_broadcast(bl_t[:], al[:])
                t1 = combp.tile([128, TCH], F32, tag="comb", name=f"t1{r}")
                t2 = combp.tile([128, TCH], F32, tag="comb", name=f"t2{r}")
                ao = aoutp.tile([128, TCH], BF16, tag="aout", name=f"ao{r}")
                nc.vector.tensor_mul(t1[:], pv_g[:], bg_t[:])
                nc.vector.tensor_mul(t2[:], pv_l[:], bl_t[:])
                nc.vector.tensor_add(ao[:], t1[:], t2[:])

                # ship finished 128-col blocks to a2a staging
                # token 1024+128c (hi) / 128c (lo) lives in chunk n at column
                # offset 128j; each unit covers 4 destination quarter-blocks
                buf = a2ai_hi if n >= 2 else a2ai_lo
                c0 = (n - 2) * 4 if n >= 2 else n * 4
                for j in range(4):
                    nc.sync.dma_start(
                        out=buf[c0 + j, h * D:(h + 1) * D, :],
                        in_=ao[:, j * 128:(j + 1) * 128])

                if n == 2 and h == 1:
                    # all-to-all #1: high-token halves (overlaps chunks 1,0)
                    nc.gpsimd.collective_compute(
                        "AllToAll", mybir.AluOpType.bypass,
                        replica_groups=[list(range(NCORES))],
                        ins=[a2ai_hi[:].opt()], outs=[a2ao_hi[:].opt()])
                    nc.sync.dma_start(
                        out=afull_hi[:],
                        in_=a2ao_hi[:].rearrange("c p n -> (c p) n")
                            .rearrange("(k p) n -> p k n", p=128))

        pssm.release()
        pspv.release()
        psqk.release()
        expp.release()
        combp.release()
        bcp.release()
        rcpp.release()
        ropet.release()
        work.release()

        # ========= phase 3: all-to-all #2 (low-token halves) =========
        nc.gpsimd.collective_compute(
            "AllToAll", mybir.AluOpType.bypass,
            replica_groups=[list(range(NCORES))],
            ins=[a2ai_lo[:].opt()], outs=[a2ao_lo[:].opt()])

        pso = tc.alloc_tile_pool(name="pso", bufs=8, space="PSUM")

        nc.sync.dma_start(
            out=afull_lo[:],
            in_=a2ao_lo[:].rearrange("c p n -> (c p) n")
                .rearrange("(k p) n -> p k n", p=128))

        # ============ phase 4: o_proj for our token slice ============
        # OUT rows 0-127 = low half-slice, rows 128-255 = high half-slice
        afulls = [afull_lo, afull_hi]
        pss2 = [[pso.tile([128, TCH], F32, tag="po", name=f"po_{tt}_{e}")
                 for e in range(NCH)] for tt in range(2)]
        for k in range(KT):
            wo_t = wop.tile([128, HID], BF16, tag="wo", name=f"wo{k}")
            nc.sync.dma_start(out=wo_t[:],
                              in_=WO[k * 128:(k + 1) * 128, :])
            for tt in range(2):
                for e in range(NCH):
                    nc.tensor.matmul(pss2[tt][e][:],
                                     afulls[tt][:, k, :],
                                     wo_t[:, e * TCH:(e + 1) * TCH],
                                     start=(k == 0), stop=(k == KT - 1))
        for tt in range(2):
            for e in range(NCH):
                ot = osb.tile([128, TCH], F32, tag="ot", name=f"ot{tt}_{e}")
                nc.vector.tensor_copy(ot[:], pss2[tt][e][:])
                nc.sync.dma_start(
                    out=OUT[tt * 128:(tt + 1) * 128,
                            e * TCH:(e + 1) * TCH],
                    in_=ot[:])
        pso.release()
        osb.release()
        wop.release()
        opool.release()
        aoutp.release()
        dram.release()
        const.release()

    nc.compile()
    return nc


def _host_prep(hidden_states, positions, k_global, v_global, w_qkv, w_o,
               w_gate, b_gate):
    """Layout-only host transforms + constant tables -> per-core in_maps."""
    f32 = np.float32
    hs = np.ascontiguousarray(np.asarray(hidden_states, f32))
    pos = np.asarray(positions)
    kg = np.asarray(k_global, f32)
    vg = np.asarray(v_global, f32)
    wqkv = np.asarray(w_qkv, f32)
    wo = np.ascontiguousarray(np.asarray(w_o, f32).astype(ml_dtypes.bfloat16))
    wg = np.asarray(w_gate, f32)
    bg = np.asarray(b_gate, f32)

    hst = np.ascontiguousarray(hs.T)

    half = D // 2
    inv_freq = (THETA ** (-np.arange(half, dtype=f32) / half)).astype(f32)
    ang = pos.astype(f32)[:, None] * inv_freq[None, :]
    cos_t = np.cos(ang).astype(f32).T       # [64, T]
    sin_t = np.sin(ang).astype(f32).T
    csf = np.ascontiguousarray(np.concatenate([cos_t, cos_t], axis=0))
    snf = np.ascontiguousarray(np.concatenate([-sin_t, sin_t], axis=0))

    p = np.arange(128, dtype=np.int64)[:, None]
    # causal diag-band base mask: CM[j][p, x] = MASKC[p, x + (3-j)*128]
    yc = np.arange(896, dtype=np.int64)[None, :]
    maskc = np.where(yc - p - 384 >= 0, 0.0, MASKV).astype(f32)
    # local band base mask: LM[jj][p, x] = MASKL[p, x + 640 - 128*jj]
    yl = np.arange(1152, dtype=np.int64)[None, :]
    dl = yl - 512 - p
    maskl = np.where((dl >= 0) & (dl <= WIN), 0.0, MASKV).astype(f32)

    ones = np.ones((128, 1), f32)
    idn = np.eye(128, dtype=f32)

    in_maps = []
    for c in range(NCORES):
        g = c // 2
        wq = wqkv[:, 2 * c * D:(2 * c + 2) * D]
        wk = wqkv[:, HQ * D + g * D:HQ * D + (g + 1) * D]
        wv = wqkv[:, (HQ + HK) * D + g * D:(HQ + HK) * D + (g + 1) * D]
        in_maps.append({
            "HST": hst,
            "WQKV": np.ascontiguousarray(np.concatenate([wq, wk, wv], axis=1)),
            "KGT": np.ascontiguousarray(kg[:, g * D:(g + 1) * D].T),
            "VG": np.ascontiguousarray(vg[:, g * D:(g + 1) * D]),
            "WO": wo,
            "WG": np.ascontiguousarray(wg[:, 2 * c:2 * c + 2]),
            "BG": np.ascontiguousarray(bg[2 * c:2 * c + 2].reshape(1, 2)),
            "CSF": csf,
            "SNF": snf,
            "ONES": ones,
            "IDN": idn,
            "MASKC": maskc,
            "MASKL": maskl,
        })
    return in_maps


def kernel(**inputs):
    if "nc" not in _CACHE:
        _CACHE["nc"] = _build()
    nc = _CACHE["nc"]
    in_maps = _host_prep(**inputs)
    res = run_bass_kernel_spmd(nc, in_maps, core_ids=list(range(NCORES)))
    out = np.empty((T, HID), np.float32)
    for c in range(NCORES):
        o = res.results[c]["OUT"]
        out[128 * c:128 * (c + 1)] = o[0:128]
        out[1024 + 128 * c:1024 + 128 * (c + 1)] = o[128:256]
    return out


# revision 29
# speedup vs baseline: 1.4075x; 1.0363x over previous
"""Trainium2 Bass kernel for LoopCoderAttention (sparse_attention).

Head-sharded tensor parallelism over 8 NeuronCores:
  core c owns query heads {2c, 2c+1} and KV head c//2.
All on-device tensors live in transposed [feature, token] layout so every
matmul contracts along the partition dim with zero on-device transposes
(except v, which needs one PE transpose per 128-tile).

o_proj: a 2MB AllToAll reshards attention output from head-sharded to
token-sharded; each core then runs the full 2048-deep contraction for its
256-token slice (the "all-reduce" happens inside the matmul accumulation).
"""
import sys
sys.path.insert(0, '/opt/trn_rl_repo')
import numpy as np
import ml_dtypes
import concourse.bass as bass
import concourse.mybir as mybir
import concourse.tile as tile
from concourse import bacc
from concourse.bass_utils import run_bass_kernel_spmd

T = 2048
HID = 2048
HQ = 16
HK = 4
D = 128
WIN = 64
THETA = 10000.0
SCALE = D ** -0.5
NCORES = 8
TCH = 512                 # t-chunk (matmul free dim)
NCH = T // TCH            # 4 chunks
KT = HID // 128           # 16 k-tiles for 2048-deep contractions
ST = T // 128             # 16 s-tiles
TSL = T // NCORES         # 256-token output slice per core
MASKV = -1e9

F32 = mybir.dt.float32
F32R = mybir.dt.float32r
BF16 = mybir.dt.bfloat16
AF = mybir.ActivationFunctionType

_CACHE = {}


def _build():
    nc = bacc.Bacc("TRN2", target_bir_lowering=False, debug=False,
                   num_devices=NCORES)
    HST = nc.dram_tensor("HST", [HID, T], F32, kind="ExternalInput").ap()
    WQKV = nc.dram_tensor("WQKV", [HID, 512], F32, kind="ExternalInput").ap()
    KGT = nc.dram_tensor("KGT", [D, T], F32, kind="ExternalInput").ap()
    VG = nc.dram_tensor("VG", [T, D], F32, kind="ExternalInput").ap()
    WO = nc.dram_tensor("WO", [HID, HID], BF16, kind="ExternalInput").ap()
    WG = nc.dram_tensor("WG", [D, 2], F32, kind="ExternalInput").ap()
    BG = nc.dram_tensor("BG", [1, 2], F32, kind="ExternalInput").ap()
    CSF = nc.dram_tensor("CSF", [128, T], F32, kind="ExternalInput").ap()
    SNF = nc.dram_tensor("SNF", [128, T], F32, kind="ExternalInput").ap()
    ONES = nc.dram_tensor("ONES", [128, 1], F32, kind="ExternalInput").ap()
    IDN = nc.dram_tensor("IDN", [128, 128], F32, kind="ExternalInput").ap()
    MASKC = nc.dram_tensor("MASKC", [128, 896], F32, kind="ExternalInput").ap()
    MASKL = nc.dram_tensor("MASKL", [128, 1152], F32, kind="ExternalInput").ap()
    OUT = nc.dram_tensor("OUT", [TSL, HID], F32, kind="ExternalOutput").ap()

    with tile.TileContext(nc) as tc:
        # pools are a strict stack: creation order is the reverse of the
        # release order at each phase boundary
        const = tc.alloc_tile_pool(name="const", bufs=1)
        dram = tc.alloc_tile_pool(name="dram", bufs=1, space="DRAM")
        aoutp = tc.alloc_tile_pool(name="aoutp", bufs=3)
        opool = tc.alloc_tile_pool(name="opool", bufs=1)
        wop = tc.alloc_tile_pool(name="wop", bufs=4)
        osb = tc.alloc_tile_pool(name="osb", bufs=2)
        work = tc.alloc_tile_pool(name="work", bufs=1)
        ropet = tc.alloc_tile_pool(name="ropet", bufs=2)
        rcpp = tc.alloc_tile_pool(name="rcpp", bufs=7)
        bcp = tc.alloc_tile_pool(name="bcp", bufs=3)
        combp = tc.alloc_tile_pool(name="combp", bufs=3)
        wqkvp = tc.alloc_tile_pool(name="wqkvp", bufs=1)
        chunkp = tc.alloc_tile_pool(name="chunkp", bufs=2)
        hsp = tc.alloc_tile_pool(name="hsp", bufs=8)
        ps1 = tc.alloc_tile_pool(name="ps1", bufs=7, space="PSUM")

        # ---- phase-1 constants first (critical path to first matmul) ----
        wqkv_sb = wqkvp.tile([128, KT, 512], F32R)
        wqkv_view = WQKV.rearrange("(k p) c -> p k c", p=128).bitcast(F32R)
        hs_first = []
        n0 = NCH - 1
        for k in range(KT):
            nc.sync.dma_start(out=wqkv_sb[:, k, :], in_=wqkv_view[:, k, :])
            if k < 8:
                hs_t = hsp.tile([128, TCH], F32R, tag="hs_t", name=f"hsf{k}")
                nc.sync.dma_start(
                    out=hs_t[:],
                    in_=HST[k * 128:(k + 1) * 128,
                            n0 * TCH:(n0 + 1) * TCH].bitcast(F32R))
                hs_first.append(hs_t)
        csf_sb = wqkvp.tile([128, T], F32)
        snf_sb = wqkvp.tile([128, T], F32)
        idn_sb = wqkvp.tile([128, 128], F32)
        wg_sb = const.tile([D, 2], F32R)
        nc.sync.dma_start(out=wg_sb[:], in_=WG.bitcast(F32R))
        bg_sb = const.tile([1, 2], F32)
        nc.sync.dma_start(out=bg_sb[:], in_=BG)
        # attention-phase constants (scheduler fills DMA idle time)
        kgt_sb = const.tile([D, T], F32R)
        vg_sb = const.tile([128, ST, D], F32R)
        ones_sb = const.tile([128, 1], F32R)
        maskc_sb = const.tile([128, 896], F32)
        maskl_sb = const.tile([128, 1152], F32)

        # ---- persistent work tiles (through attention) ----
        qrot = work.tile([128, 2, T], F32R)
        krot = work.tile([128, T], F32R)
        vcur = work.tile([128, ST, D], F32R)   # current v in [s, d] tiles
        gate = work.tile([8, TCH], F32)        # row 2n+h (DMA-staged access)

        a2ai_hi = dram.tile([NCORES, 2 * D, TSL // 2], BF16)
        a2ao_hi = dram.tile([NCORES, 2 * D, TSL // 2], BF16)
        a2ai_lo = dram.tile([NCORES, 2 * D, TSL // 2], BF16)
        a2ao_lo = dram.tile([NCORES, 2 * D, TSL // 2], BF16)

        def rope_chunk(dst_full, src, n):
            """dst_full[:, n*TCH:...] = neox-rope of chunk tile src [128, TCH].

            rot = src * [cos;cos] + rot90(src) * [-sin;sin], where rot90 swaps
            the two 64-partition halves (built with two SBUF->SBUF DMAs since
            DVE ops require matching base partitions).
            """
            sl = bass.ds(n * TCH, TCH)
            sr = ropet.tile([128, TCH], F32, tag="ropesr", name=f"sr{n}")
            nc.sync.dma_start(out=sr[0:64, :], in_=src[64:128, :])
            nc.sync.dma_start(out=sr[64:128, :], in_=src[0:64, :])
            ta = ropet.tile([128, TCH], F32, tag="ropetmp", name=f"ra{n}")
            tb = ropet.tile([128, TCH], F32, tag="ropetmp", name=f"rb{n}")
            nc.vector.tensor_mul(ta[:], src[:], csf_sb[:, sl])
            nc.vector.tensor_mul(tb[:], sr[:], snf_sb[:, sl])
            nc.vector.tensor_add(dst_full[:, sl], ta[:], tb[:])

        nc.sync.dma_start(out=csf_sb[:], in_=CSF)
        nc.sync.dma_start(out=snf_sb[:], in_=SNF)
        nc.sync.dma_start(out=idn_sb[:], in_=IDN)

        # ================= phase 1: qkvT = wqkv^T @ hsT =================
        for n in reversed(range(NCH)):
            pss = [ps1.tile([128, TCH], F32, tag="ps1t", name=f"ps1_{n}_{m}")
                   for m in range(4)]
            for k in range(KT):
                if n == NCH - 1 and k < 8:
                    hs_t = hs_first[k]
                else:
                    hs_t = hsp.tile([128, TCH], F32R, tag="hs_t",
                                    name=f"hs_{n}_{k}")
                    nc.sync.dma_start(
                        out=hs_t[:],
                        in_=HST[k * 128:(k + 1) * 128,
                                n * TCH:(n + 1) * TCH].bitcast(F32R))
                for m in range(4):
                    nc.tensor.matmul(pss[m][:],
                                     wqkv_sb[:, k, m * 128:(m + 1) * 128],
                                     hs_t[:],
                                     start=(k == 0), stop=(k == KT - 1))
            sl = bass.ds(n * TCH, TCH)
            q0c = chunkp.tile([128, TCH], F32, tag="q0c")
            q1c = chunkp.tile([128, TCH], F32, tag="q1c")
            kc = chunkp.tile([128, TCH], F32, tag="kc")
            vc = chunkp.tile([128, TCH], F32, tag="vc")
            nc.scalar.activation(q0c[:], pss[0][:], AF.Copy)
            nc.scalar.activation(q1c[:], pss[1][:], AF.Copy)
            nc.scalar.activation(kc[:], pss[2][:], AF.Copy)
            nc.vector.tensor_copy(vc[:], pss[3][:])

            rope_chunk(qrot[:, 0, :], q0c, n)
            rope_chunk(qrot[:, 1, :], q1c, n)
            rope_chunk(krot, kc, n)

            # transpose v tiles of this chunk: vcur[s] = vc[:, j*128:...]^T
            for j in range(4):
                s = 4 * n + j
                pt = ps1.tile([128, 128], F32, tag="ps1g", name=f"pt{s}", bufs=1)
                nc.tensor.transpose(pt[:], vc[:, j * 128:(j + 1) * 128],
                                    idn_sb[:])
                nc.vector.tensor_copy(vcur[:, s, :], pt[:])

            # gate for this chunk (both heads)
            for h in range(2):
                r = 2 * n + h
                gp = ps1.tile([1, TCH], F32, tag="ps1g", name=f"gp{r}", bufs=1)
                nc.tensor.matmul(gp[:], wg_sb[:, h:h + 1], qrot[:, h, sl],
                                 start=True, stop=True)
                gst = chunkp.tile([1, TCH], F32, tag="gst", name=f"gst{r}")
                nc.scalar.activation(gst[:], gp[:], AF.Sigmoid,
                                     bias=bg_sb[0:1, h:h + 1])
                nc.sync.dma_start(out=gate[r:r + 1, :], in_=gst[:])

        nc.sync.dma_start(out=kgt_sb[:], in_=KGT.bitcast(F32R))
        nc.sync.dma_start(out=vg_sb[:],
                          in_=VG.rearrange("(s p) d -> p s d", p=128).bitcast(F32R))
        nc.sync.dma_start(out=ones_sb[:], in_=ONES.bitcast(F32R))
        nc.sync.dma_start(out=maskc_sb[:], in_=MASKC)
        nc.sync.dma_start(out=maskl_sb[:], in_=MASKL)

        ps1.release()
        hsp.release()
        chunkp.release()
        wqkvp.release()

        afull_hi = opool.tile([128, KT, TSL // 2], BF16)
        afull_lo = opool.tile([128, KT, TSL // 2], BF16)

        expp = tc.alloc_tile_pool(name="expp", bufs=6)
        psqk = tc.alloc_tile_pool(name="psqk", bufs=4, space="PSUM")
        pspv = tc.alloc_tile_pool(name="pspv", bufs=3, space="PSUM")
        pssm = tc.alloc_tile_pool(name="pssm", bufs=1, space="PSUM")

        # ============ phase 2: attention (global + local) ============
        # chunks descend so the high-token half finishes first and its
        # all-to-all overlaps the low-token half's compute
        for n in reversed(range(NCH)):
            for h in range(2):
                sl = bass.ds(n * TCH, TCH)
                q_ap = qrot[:, h, sl]
                r = 2 * n + h
                gsl_t = rcpp.tile([1, TCH], F32, tag="rcp", name=f"gsl{r}")
                nc.sync.dma_start(out=gsl_t[:], in_=gate[r:r + 1, :])
                gsl = gsl_t[:]
                g1 = rcpp.tile([1, TCH], F32, tag="rcp", name=f"g1{r}")
                nc.vector.tensor_scalar(g1[:], gsl, -1.0, 1.0,
                                        mybir.AluOpType.mult,
                                        mybir.AluOpType.add)

                def pass_(kT_ap, v_ap, s_tiles, mask_ap_of, pfx):
                    pv = pspv.tile([128, TCH], F32, tag="pv", name=f"pv{pfx}")
                    sm = pssm.tile([1, TCH], F32, tag="sm", name=f"sm{pfx}")
                    first = True
                    for s in s_tiles:
                        qk = psqk.tile([128, TCH], F32, tag="qk",
                                       name=f"qk{pfx}_{s}")
                        nc.tensor.matmul(qk[:], kT_ap[:, s * 128:(s + 1) * 128],
                                         q_ap, start=True, stop=True)
                        m_ap = mask_ap_of(s)
                        if m_ap is not None:
                            nc.vector.tensor_add(qk[:], qk[:], m_ap)
                        ex = expp.tile([128, TCH], F32R, tag="ex", name=f"ex{pfx}_{s}")
                        nc.scalar.activation(ex[:], qk[:], AF.Exp, scale=SCALE)
                        last = (s == s_tiles[-1])
                        nc.tensor.matmul(pv[:], v_ap[:, s, :], ex[:],
                                         start=first, stop=last)
                        nc.tensor.matmul(sm[:], ones_sb[:], ex[:],
                                         start=first, stop=last)
                        first = False
                    return pv, sm

                # global pass over cached KV: causal mask on diagonal tiles
                gs = list(range(0, 4 * n + 4))

                def gmask(s, n=n):
                    j = s - 4 * n
                    if j < 0:
                        return None
                    off = (3 - j) * 128
                    return maskc_sb[:, off:off + TCH]

                pv_g, sm_g = pass_(kgt_sb, vg_sb, gs, gmask, f"g{h}{n}")

                # local pass over current KV: sliding-window band masks
                ls = [s for s in range(4 * n - 1, 4 * n + 4) if s >= 0]

                def lmask(s, n=n):
                    jj = s - (4 * n - 1)
                    off = 640 - 128 * jj
                    return maskl_sb[:, off:off + TCH]

                pv_l, sm_l = pass_(krot, vcur, ls, lmask, f"l{h}{n}")

                # drain sums fast (frees pssm); pv stays in psum until combine
                sg = rcpp.tile([1, TCH], F32, tag="rcp", name=f"sg{r}")
                sl_ = rcpp.tile([1, TCH], F32, tag="rcp", name=f"sl{r}")
                nc.scalar.activation(sg[:], sm_g[:], AF.Copy)
                nc.scalar.activation(sl_[:], sm_l[:], AF.Copy)
                # a_g = gate/sum_g ; a_l = (1-gate)/sum_l
                ag = rcpp.tile([1, TCH], F32, tag="rcp", name=f"ag{r}")
                al = rcpp.tile([1, TCH], F32, tag="rcp", name=f"al{r}")
                rg = rcpp.tile([1, TCH], F32, tag="rcp", name=f"rg{r}")
                rl = rcpp.tile([1, TCH], F32, tag="rcp", name=f"rl{r}")
                nc.vector.reciprocal_approx_fast(rg[:], sg[:])
                nc.vector.reciprocal_approx_fast(rl[:], sl_[:])
                nc.vector.tensor_mul(ag[:], gsl, rg[:])
                nc.vector.tensor_mul(al[:], g1[:], rl[:])
                bg_t = bcp.tile([128, TCH], F32, tag="bcast", name=f"bg_t{r}")
                bl_t = bcp.tile([128, TCH], F32, tag="bcast", name=f"bl_t{r}")
                nc.gpsimd.partition_broadcast(bg_t[:], ag[:])
                nc.gpsimd.partition_broadcast(bl_t[:], al[:])
                t1 = combp.tile([128, TCH], F32, tag="comb", name=f"t1{r}")
                t2 = combp.tile([128, TCH], F32, tag="comb", name=f"t2{r}")
                ao = aoutp.tile([128, TCH], BF16, tag="aout", name=f"ao{r}")
                nc.vector.tensor_mul(t1[:], pv_g[:], bg_t[:])
                nc.vector.tensor_mul(t2[:], pv_l[:], bl_t[:])
                nc.vector.tensor_add(ao[:], t1[:], t2[:])

                # ship finished 128-col blocks to a2a staging
                # token 1024+128c (hi) / 128c (lo) lives in chunk n at column
                # offset 128j; each unit covers 4 destination quarter-blocks
                buf = a2ai_hi if n >= 2 else a2ai_lo
                c0 = (n - 2) * 4 if n >= 2 else n * 4
                for j in range(4):
                    nc.sync.dma_start(
                        out=buf[c0 + j, h * D:(h + 1) * D, :],
                        in_=ao[:, j * 128:(j + 1) * 128])

                if n == 2 and h == 1:
                    # all-to-all #1: high-token halves (overlaps chunks 1,0)
                    nc.gpsimd.collective_compute(
                        "AllToAll", mybir.AluOpType.bypass,
                        replica_groups=[list(range(NCORES))],
                        ins=[a2ai_hi[:].opt()], outs=[a2ao_hi[:].opt()])
                    nc.sync.dma_start(
                        out=afull_hi[:],
                        in_=a2ao_hi[:].rearrange("c p n -> (c p) n")
                            .rearrange("(k p) n -> p k n", p=128))

        pssm.release()
        pspv.release()
        psqk.release()
        expp.release()
        combp.release()
        bcp.release()
        rcpp.release()
        ropet.release()
        work.release()

        # ========= phase 3: all-to-all #2 (low-token halves) =========
        nc.gpsimd.collective_compute(
            "AllToAll", mybir.AluOpType.bypass,
            replica_groups=[list(range(NCORES))],
            ins=[a2ai_lo[:].opt()], outs=[a2ao_lo[:].opt()])

        pso = tc.alloc_tile_pool(name="pso", bufs=8, space="PSUM")

        nc.sync.dma_start(
            out=afull_lo[:],
            in_=a2ao_lo[:].rearrange("c p n -> (c p) n")
                .rearrange("(k p) n -> p k n", p=128))

        # ============ phase 4: o_proj for our token slice ============
        # OUT rows 0-127 = low half-slice, rows 128-255 = high half-slice
        afulls = [afull_lo, afull_hi]
        pss2 = [[pso.tile([128, TCH], F32, tag="po", name=f"po_{tt}_{e}")
                 for e in range(NCH)] for tt in range(2)]
        for k in range(KT):
            wo_t = wop.tile([128, HID], BF16, tag="wo", name=f"wo{k}")
            nc.sync.dma_start(out=wo_t[:],
                              in_=WO[k * 128:(k + 1) * 128, :])
            for tt in range(2):
                for e in range(NCH):
                    nc.tensor.matmul(pss2[tt][e][:],
                                     afulls[tt][:, k, :],
                                     wo_t[:, e * TCH:(e + 1) * TCH],
                                     start=(k == 0), stop=(k == KT - 1))
        for tt in range(2):
            for e in range(NCH):
                ot = osb.tile([128, TCH], F32, tag="ot", name=f"ot{tt}_{e}")
                nc.vector.tensor_copy(ot[:], pss2[tt][e][:])
                nc.sync.dma_start(
                    out=OUT[tt * 128:(tt + 1) * 128,
                            e * TCH:(e + 1) * TCH],
                    in_=ot[:])
        pso.release()
        osb.release()
        wop.release()
        opool.release()
        aoutp.release()
        dram.release()
        const.release()

    nc.compile()
    return nc


def _host_prep(hidden_states, positions, k_global, v_global, w_qkv, w_o,
               w_gate, b_gate):
    """Layout-only host transforms + constant tables -> per-core in_maps."""
    f32 = np.float32
    hs = np.ascontiguousarray(np.asarray(hidden_states, f32))
    pos = np.asarray(positions)
    kg = np.asarray(k_global, f32)
    vg = np.asarray(v_global, f32)
    wqkv = np.asarray(w_qkv, f32)
    wo = np.ascontiguousarray(np.asarray(w_o, f32).astype(ml_dtypes.bfloat16))
    wg = np.asarray(w_gate, f32)
    bg = np.asarray(b_gate, f32)

    hst = np.ascontiguousarray(hs.T)

    half = D // 2
    inv_freq = (THETA ** (-np.arange(half, dtype=f32) / half)).astype(f32)
    ang = pos.astype(f32)[:, None] * inv_freq[None, :]
    cos_t = np.cos(ang).astype(f32).T       # [64, T]
    sin_t = np.sin(ang).astype(f32).T
    csf = np.ascontiguousarray(np.concatenate([cos_t, cos_t], axis=0))
    snf = np.ascontiguousarray(np.concatenate([-sin_t, sin_t], axis=0))

    p = np.arange(128, dtype=np.int64)[:, None]
    # causal diag-band base mask: CM[j][p, x] = MASKC[p, x + (3-j)*128]
    yc = np.arange(896, dtype=np.int64)[None, :]
    maskc = np.where(yc - p - 384 >= 0, 0.0, MASKV).astype(f32)
    # local band base mask: LM[jj][p, x] = MASKL[p, x + 640 - 128*jj]
    yl = np.arange(1152, dtype=np.int64)[None, :]
    dl = yl - 512 - p
    maskl = np.where((dl >= 0) & (dl <= WIN), 0.0, MASKV).astype(f32)

    ones = np.ones((128, 1), f32)
    idn = np.eye(128, dtype=f32)

    in_maps = []
    for c in range(NCORES):
        g = c // 2
        wq = wqkv[:, 2 * c * D:(2 * c + 2) * D]
        wk = wqkv[:, HQ * D + g * D:HQ * D + (g + 1) * D]
        wv = wqkv[:, (HQ + HK) * D + g * D:(HQ + HK) * D + (g + 1) * D]
        in_maps.append({
            "HST": hst,
            "WQKV": np.ascontiguousarray(np.concatenate([wq, wk, wv], axis=1)),
            "KGT": np.ascontiguousarray(kg[:, g * D:(g + 1) * D].T),
            "VG": np.ascontiguousarray(vg[:, g * D:(g + 1) * D]),
            "WO": wo,
            "WG": np.ascontiguousarray(wg[:, 2 * c:2 * c + 2]),
            "BG": np.ascontiguousarray(bg[2 * c:2 * c + 2].reshape(1, 2)),
            "CSF": csf,
            "SNF": snf,
            "ONES": ones,
            "IDN": idn,
            "MASKC": maskc,
            "MASKL": maskl,
        })
    return in_maps


def kernel(**inputs):
    if "nc" not in _CACHE:
        _CACHE["nc"] = _build()
    nc = _CACHE["nc"]
    in_maps = _host_prep(**inputs)
    res = run_bass_kernel_spmd(nc, in_maps, core_ids=list(range(NCORES)))
    out = np.empty((T, HID), np.float32)
    for c in range(NCORES):
        o = res.results[c]["OUT"]
        out[128 * c:128 * (c + 1)] = o[0:128]
        out[1024 + 128 * c:1024 + 128 * (c + 1)] = o[128:256]
    return out
